# revision 102
# baseline (speedup 1.0000x reference)
"""Trainium2 Bass kernel for nn_CausalSelfAttention_35931696398729.

Sharding: 8 cores = (batch b in {0,1}) x (kv-head n in {0..3}).
Each core computes its 4 query heads' causal GQA attention for its batch
plus the partial c_proj (rows of Wo for its heads); the host sums the 4
partials per batch.  No device collectives.

Layouts are "transposed" throughout: qT/kT (d on partitions, t free) so
scores come out as ST (keys on partitions, queries free) and PV consumes
exp(ST) directly;  c_proj consumes the attention output OT (d, t) as the
stationary operand with no transposes anywhere except V (16 PE-transposes).

The projections and c_proj run as fp8e4m3 DoubleRow matmuls on hi/lo
split operands (A ~= A_hi + A_lo, both fp8): the product uses the three
dominant terms Ah*Bh + Al*Bh + Ah*Bl, which is ~11 mantissa bits of
effective precision (better than bf16) at 0.75x the bf16 PE cost, since
each DoubleRow instruction contracts two 128-deep groups at 0.5
cycles/row.  Weights are pre-scaled by WS=64 on the host so their hi/lo
parts stay in e4m3's normal range; the 1/WS descale rides the psum
drains.  Attention (QK, exp, PV) stays bf16.

QK RMSNorm is folded in without normalizing q/k tensors elementwise:
 - q-side factor r_q(t)/sqrt(HD) multiplies qT columns (query temperature)
 - k-side factor r_k(s) rides the Exp activation's per-partition scale
 - gamma_q*gamma_k multiplies kT rows (per-partition)
 - softmax runs without max-subtraction (|scores| <= sqrt(HD) after norm)
 - 1/rowsum is applied to OT columns after PV.
"""

import sys
from contextlib import ExitStack

sys.path.insert(0, "/opt/trn_rl_repo")

import ml_dtypes
import numpy as np

import concourse.bacc as bacc
import concourse.mybir as mybir
import concourse.tile as tile
from concourse import bass_utils
from concourse.masks import make_identity

B, T, D = 2, 2048, 2048
NH, NKV, HD = 16, 4, 128
G = NH // NKV  # query heads per core
EPS = 1e-6
THETA = 10000.0
N_CORES = 8
P = 128
TC = 512            # q-chunk for attention / c_proj column chunk
NTC = T // TC       # 4
TC1 = 256           # t-chunk for phase-1 projections
NTC1 = T // TC1     # 8
NKT = D // P        # 16 contraction chunks
NKP = NKT // 2      # 8 DoubleRow contraction pairs
NTB = T // P        # 16 t-blocks

WS = 64.0           # host-side weight prescale for fp8 ranges
F32 = mybir.dt.float32
F8 = mybir.dt.float8e4
ST_DT = mybir.dt.bfloat16
NP_ST = ml_dtypes.bfloat16
NP_F8 = ml_dtypes.float8_e4m3
DR = mybir.MatmulPerfMode.DoubleRow

# (w_level, x_level) term order for the 3-term hi/lo product; the w_lo
# pass runs last so each weight tensor's lo half may arrive after its hi.
L3_TERMS = ((0, 0), (0, 1), (1, 0))


def build_program():
    nc = bacc.Bacc("TRN2", target_bir_lowering=False, debug=False,
                   enable_asserts=False, num_devices=N_CORES)

    y_dt = ST_DT
    # inputs arrive host-prepacked in SBUF layout (partition-major), with
    # fp8 hi/lo splits precomputed on the host.
    xT8 = nc.dram_tensor("xT8", (P, NTC1, 2, NKT, TC1), F8,
                         kind="ExternalInput").ap()
    wq8 = nc.dram_tensor("wq8", (P, 2, G, NKT, HD), F8,
                         kind="ExternalInput").ap()
    wk8 = nc.dram_tensor("wk8", (P, 2, NKT, HD), F8,
                         kind="ExternalInput").ap()
    wv8 = nc.dram_tensor("wv8", (P, 2, NKT, HD), F8,
                         kind="ExternalInput").ap()
    wo8 = nc.dram_tensor("wo8", (P, 2, G, D), F8,
                         kind="ExternalInput").ap()
    tabs = nc.dram_tensor("tabs", (P, 2, T), ST_DT,
                          kind="ExternalInput").ap()
    gamma2 = nc.dram_tensor("gamma2", (P, 1), F32, kind="ExternalInput").ap()
    y = nc.dram_tensor("y", (T, D), y_dt, kind="ExternalOutput").ap()

    with tile.TileContext(nc) as tc, \
         nc.allow_low_precision(reason="fp8/bf16 matmul operand tiles"):
        with tc.tile_pool(name="persist", bufs=1) as persist, \
             tc.tile_pool(name="stri2", bufs=2) as stri2, \
             tc.tile_pool(name="weights", bufs=1) as wpool, \
             tc.tile_pool(name="xts", bufs=4) as xpool, \
             tc.tile_pool(name="p1tmp", bufs=3) as tmpool:
            tab_sb = persist.tile([P, 2, T], ST_DT)
            cos_sb = tab_sb[:, 0, :]
            sin_sb = tab_sb[:, 1, :]
            g2_sb = persist.tile([P, 1], F32)
            ident_f32 = persist.tile([P, P], F32)
            make_identity(nc, ident_f32)
            ident = persist.tile([P, P], ST_DT)
            nc.vector.tensor_copy(out=ident, in_=ident_f32)
            ones_f32 = persist.tile([P, P], F32)
            nc.vector.memset(ones_f32, 1.0)
            ones_col = persist.tile([P, 1], ST_DT)
            nc.vector.tensor_copy(out=ones_col, in_=ones_f32[:, 0:1])
            invws = persist.tile([P, 1], F32)
            nc.vector.memset(invws, 1.0 / WS)
            eps_k = persist.tile([P, 1], F32)
            nc.vector.memset(eps_k, EPS)
            eps_q = persist.tile([1, 1], F32)
            nc.vector.memset(eps_q, HD * EPS)
            # causal triangle mask (keep iff col >= row), built once; the
            # per-diagonal-block masking is then a cheap 2x-mode DVE multiply
            # instead of a Pool affine_select in the exp->PV chain
            tri = persist.tile([P, P], ST_DT)
            nc.vector.memset(tri, 1.0)
            nc.gpsimd.affine_select(out=tri, in_=tri, pattern=[[1, P]],
                                    compare_op=mybir.AluOpType.is_ge,
                                    fill=0.0, base=0, channel_multiplier=-1)

            q_sb = [persist.tile([P, T], ST_DT, tag=f"q_sb{h}", name=f"q_sb{h}")
                    for h in range(G)]
            kT_sb = persist.tile([P, T], ST_DT)
            v_sb = persist.tile([P, NTB, P], ST_DT)
            rk_tiles = persist.tile([P, NTB], F32)

            # ---------------- Phase 1: projections + RoPE + norms -----------
            xts_tiles = {}
            sqk_tiles = {}
            with tc.tile_pool(name="p1ps", bufs=3, space="PSUM") as ps_a, \
                 tc.tile_pool(name="p1psv", bufs=1, space="PSUM") as ps_v, \
                 tc.tile_pool(name="p1str", bufs=1, space="PSUM") as ps_s:
                wq_sb = wpool.tile([P, 2, G, NKT, HD], F8)
                wk_sb = wpool.tile([P, 2, NKT, HD], F8)
                wv_sb = wpool.tile([P, 2, NKT, HD], F8)

                def proj_mms(ps, w_sb, xts):
                    """24 DoubleRow matmuls: 3-term hi/lo product into ps."""
                    first = True
                    for wl, xl in L3_TERMS:
                        for kp in range(NKP):
                            nc.tensor.matmul(
                                ps, w_sb[:, wl, 2 * kp:2 * kp + 2, :],
                                xts[:, xl, 2 * kp:2 * kp + 2, :],
                                start=first,
                                stop=(wl, xl) == L3_TERMS[-1]
                                and kp == NKP - 1,
                                perf_mode=DR)
                            first = False

                def rk_block(ci):
                    # rk column tiles for chunk ci's key blocks:
                    # 1/sqrt(colsum(sq_k)/HD + eps); runs a full chunk after
                    # the square so the PE queue never stalls on it (and so
                    # the tabs DMA may arrive as late as chunk 1)
                    sq_k = sqk_tiles[ci]
                    for i in range(TC1 // P):
                        kb = ci * (TC1 // P) + i
                        ssqc = ps_s.tile([P, 1], F32, tag="ssqc",
                                         name="ssqc")
                        nc.tensor.matmul(ssqc, sq_k[:, i * P:(i + 1) * P],
                                         ones_f32[:, 0:1],
                                         start=True, stop=True)
                        nc.scalar.activation(
                            out=rk_tiles[:, kb:kb + 1], in_=ssqc,
                            func=mybir.ActivationFunctionType.Sqrt,
                            bias=eps_k[:], scale=float(1.0 / HD))
                        nc.vector.reciprocal(out=rk_tiles[:, kb:kb + 1],
                                             in_=rk_tiles[:, kb:kb + 1])

                def load_x(i):
                    xts = xpool.tile([P, 2, NKT, TC1], F8, tag="xts",
                                     name="xts")
                    xts_tiles[i] = xts
                    nc.sync.dma_start(out=xts, in_=xT8[:, i])

                # startup staging, ordered by when PE consumes each transfer
                # (HWDGE serializes dma_starts at ~625ns each): K weights and
                # x0 hi halves first so the first DoubleRow pass can start,
                # then x0 lo (pass 2), wk lo (pass 3), chunk-0 rope tables,
                # per-head q weights (hi then lo, matching the pass order),
                # v weights, then x and table chunks just-in-time.
                xts0 = xpool.tile([P, 2, NKT, TC1], F8, tag="xts",
                                  name="xts")
                xts_tiles[0] = xts0
                # the very first DoubleRow needs only the kt 0-1 slices of
                # wk-hi and x0-hi: land those in two tiny transfers so PE
                # starts ~1us earlier, then stream the rest
                nc.sync.dma_start(out=wk_sb[:, 0, 0:2, :],
                                  in_=wk8[:, 0, 0:2, :])
                nc.sync.dma_start(out=xts0[:, 0, 0:2, :],
                                  in_=xT8[:, 0, 0, 0:2, :])
                nc.sync.dma_start(out=wk_sb[:, 0, 2:, :],
                                  in_=wk8[:, 0, 2:, :])
                nc.sync.dma_start(out=xts0[:, 0, 2:NKP, :],
                                  in_=xT8[:, 0, 0, 2:NKP, :])
                nc.sync.dma_start(out=xts0[:, 0, NKP:, :],
                                  in_=xT8[:, 0, 0, NKP:, :])
                nc.sync.dma_start(out=xts0[:, 1], in_=xT8[:, 0, 1])
                nc.sync.dma_start(out=wk_sb[:, 1], in_=wk8[:, 1])
                for h in range(G):
                    nc.sync.dma_start(out=wq_sb[:, 0, h], in_=wq8[:, 0, h])
                    nc.sync.dma_start(out=wq_sb[:, 1, h], in_=wq8[:, 1, h])
                    if h == 0:
                        # chunk-0 rope tables: not consumed until the first
                        # rope DVE (~10us), so they ride behind head 0's
                        # weights instead of delaying them
                        nc.sync.dma_start(out=tab_sb[:, :, 0:TC1],
                                          in_=tabs[:, :, 0:TC1])
                        nc.sync.dma_start(out=g2_sb, in_=gamma2)
                nc.sync.dma_start(out=wv_sb, in_=wv8)
                load_x(1)
                nc.sync.dma_start(out=tab_sb[:, :, TC1:2 * TC1],
                                  in_=tabs[:, :, TC1:2 * TC1])
                load_x(2)
                load_x(3)
                nc.sync.dma_start(out=tab_sb[:, :, 2 * TC1:4 * TC1],
                                  in_=tabs[:, :, 2 * TC1:4 * TC1])
                nc.sync.dma_start(out=tab_sb[:, :, 4 * TC1:],
                                  in_=tabs[:, :, 4 * TC1:])

                qnorm_tails = {}
                sq_lists = {}
                rq_lists = {}

                def ssq_mm(ci, h):
                    # PE column-sum for chunk ci head h, then sqrt/recip
                    # immediately (ACT / DVE are free here); runs a chunk
                    # after the square so the PE queue never stalls on it
                    ssq = ps_s.tile([1, TC1], F32, tag="ssq",
                                    name="ssq_q", bufs=2)
                    nc.tensor.matmul(ssq, ones_col,
                                     sq_lists[ci][h],
                                     start=True, stop=True)
                    sq_s = stri2.tile([1, TC1], F32, tag="sqs",
                                      name="sq_sq", bufs=4)
                    nc.scalar.activation(
                        out=sq_s, in_=ssq,
                        func=mybir.ActivationFunctionType.Sqrt,
                        bias=eps_q[:], scale=1.0)
                    rq_row = stri2.tile([1, TC1], ST_DT, tag="rqrow",
                                        name="rq_row", bufs=4)
                    nc.vector.reciprocal(out=rq_row, in_=sq_s)
                    rq_lists[ci].append(rq_row)

                for tc_i in range(NTC1):
                    sl = slice(tc_i * TC1, (tc_i + 1) * TC1)
                    qnorm_tail = qnorm_tails.setdefault(tc_i, [])
                    sq_lists[tc_i] = []
                    rq_lists[tc_i] = []
                    if tc_i >= 4:
                        load_x(tc_i)
                    xts = xts_tiles[tc_i]
                    # all 6 projections of the chunk drain (via ACT, to
                    # bf16, descaled by 1/WS) into one batch tile; one DMA
                    # pair then builds a half-swapped copy so every rope DVE
                    # op below is all-SBUF bf16 (2x mode, aligned bases)
                    psb_all = tmpool.tile([P, 6, TC1], ST_DT, tag="pall",
                                          name="pall", bufs=2)

                    # ---- K ----
                    ps = ps_a.tile([P, TC1], F32, tag="proj", name="ps_k")
                    proj_mms(ps, wk_sb, xts)
                    nc.scalar.mul(psb_all[:, 0, :], ps, 1.0 / WS)
                    # RoPE is a rotation, so per-token norms are the same
                    # before and after it: the norm squares read the raw
                    # projection drain, decoupling the whole sqrt/recip
                    # chain from the serial DVE rope tail.
                    sqt = tmpool.tile([P, TC1], F32, tag="ropesq",
                                      name="ropesq")
                    nc.scalar.square(out=sqt, in_=psb_all[:, 0, :])
                    sqk_tiles[tc_i] = sqt

                    # ---- Q heads (prev chunk's norm PE bits interleave) ----
                    for h in range(G):
                        ps = ps_a.tile([P, TC1], F32, tag="proj",
                                       name="ps_q")
                        proj_mms(ps, wq_sb[:, :, h], xts)
                        nc.scalar.mul(psb_all[:, 1 + h, :], ps, 1.0 / WS)
                        sq_q = tmpool.tile([P, TC1], ST_DT, tag="qsq",
                                           name="sq_q", bufs=5)
                        nc.scalar.square(out=sq_q, in_=psb_all[:, 1 + h, :])
                        sq_lists[tc_i].append(sq_q)
                        if tc_i > 0:
                            ssq_mm(tc_i - 1, h)
                            if h == 0:
                                rk_block(tc_i - 1)

                    # ---- V (chunks 4-7 run at the phase boundary) ----
                    if tc_i < 4:
                        ps = ps_a.tile([P, TC1], F32, tag="proj",
                                       name="ps_vp")
                        proj_mms(ps, wv_sb, xts)
                        nc.scalar.mul(psb_all[:, 5, :], ps, 1.0 / WS)

                    # ---- batched half-swap ----
                    psw_all = tmpool.tile([P, 6, TC1], ST_DT, tag="pswp",
                                          name="pswp", bufs=2)
                    nc.sync.dma_start(out=psw_all[0:64, :, :],
                                      in_=psb_all[64:128, :, :])
                    nc.sync.dma_start(out=psw_all[64:128, :, :],
                                      in_=psb_all[0:64, :, :])

                    def rope_batch(dst, j):
                        tmp = tmpool.tile([P, TC1], ST_DT, tag="ropetmp",
                                          name="ropetmp")
                        nc.vector.tensor_mul(out=tmp,
                                             in0=psw_all[:, j, :],
                                             in1=sin_sb[:, sl])
                        tmp2 = tmpool.tile([P, TC1], ST_DT, tag="ropetmp2",
                                           name="ropetmp2")
                        nc.vector.tensor_mul(out=tmp2,
                                             in0=psb_all[:, j, :],
                                             in1=cos_sb[:, sl])
                        nc.vector.tensor_add(out=dst[:, sl], in0=tmp2,
                                             in1=tmp)

                    rope_batch(kT_sb, 0)
                    # gamma2 applied after the (pre-rope) norm-square
                    nc.vector.tensor_scalar_mul(out=kT_sb[:, sl],
                                                in0=kT_sb[:, sl],
                                                scalar1=g2_sb)
                    for h in range(G):
                        rope_batch(q_sb[h], 1 + h)

                        def qnorm(h=h, sl=sl, ci=tc_i):
                            # rq row -> all partitions on Pool (PE is the
                            # bottleneck; Pool idles in phase 1)
                            rb_sb = tmpool.tile([P, TC1], ST_DT, tag="rqb",
                                                name="rb_sb", bufs=2)
                            nc.gpsimd.partition_broadcast(
                                rb_sb, rq_lists[ci][h])
                            nc.vector.tensor_mul(out=q_sb[h][:, sl],
                                                 in0=q_sb[h][:, sl],
                                                 in1=rb_sb)

                        qnorm_tail.append(qnorm)

                    # PE transposes of V read the batch tile directly
                    if tc_i < 4:
                        for i in range(TC1 // P):
                            pst = ps_v.tile([P, P], ST_DT, tag="vtr",
                                            name="pst")
                            nc.tensor.transpose(
                                pst, psb_all[:, 5, i * P:(i + 1) * P],
                                ident)
                            nc.vector.tensor_copy(
                                out=v_sb[:, tc_i * (TC1 // P) + i, :],
                                in_=pst)
                    # previous chunk's temper finalizers
                    for fn_ in qnorm_tails.get(tc_i - 1, []):
                        fn_()
                    if tc_i == NTC1 - 1:
                        # chunk 7's norm tail runs here so every Sqrt
                        # activation precedes the first phase-2 Exp in the
                        # schedule (one act-table switch, not a thrash)
                        for h in range(G):
                            ssq_mm(tc_i, h)
                        rk_block(tc_i)
                        for fn_ in qnorm_tails[tc_i]:
                            fn_()

                # V projections for chunks 4-7 are deferred into phase 2 as
                # qc0 filler ops (see v_tail_ops below).

                # whole-tile copy of the exp scales: phase-2 exps read this
                # copy, so they depend on EVERY rk column (not just their
                # own kb slice) and the scheduler cannot hoist the first
                # exps in between chunk 7's Sqrt chains on the ACT queue --
                # which would thrash the activation-function table (no set
                # holds both Sqrt and Exp).
                rk_all = persist.tile([P, NTB], F32)
                nc.vector.tensor_copy(out=rk_all, in_=rk_tiles)

            # ---------------- Phase 2: attention ---------------------------
            with ExitStack() as p2stack:
                wopool = p2stack.enter_context(
                    tc.tile_pool(name="wo", bufs=1))
                apool = p2stack.enter_context(
                    tc.tile_pool(name="attn", bufs=2))
                ppool = p2stack.enter_context(
                    tc.tile_pool(name="psb", bufs=6))
                otpool = p2stack.enter_context(
                    tc.tile_pool(name="otn", bufs=1))
                wo_sb = wopool.tile([P, 2, G, D], F8)
                for h in range(G):
                    nc.sync.dma_start(out=wo_sb[:, 0, h], in_=wo8[:, 0, h])
                for h in range(G):
                    nc.sync.dma_start(out=wo_sb[:, 1, h], in_=wo8[:, 1, h])
                # hi/lo fp8 attention output, heads adjacent per level so a
                # DoubleRow lhsT can pair two heads' d-blocks
                otn_sb = otpool.tile([P, 2, G, T], F8)

                with ExitStack() as psstack:
                    ps_st = psstack.enter_context(
                        tc.tile_pool(name="p2st", bufs=2, space="PSUM"))
                    ps_ot = psstack.enter_context(
                        tc.tile_pool(name="p2ot", bufs=2, space="PSUM"))
                    ps_rs = psstack.enter_context(
                        tc.tile_pool(name="p2rs", bufs=1, space="PSUM"))
                    ps_rw = psstack.enter_context(
                        tc.tile_pool(name="p2rw", bufs=1, space="PSUM"))
                    ps_y = psstack.enter_context(
                        tc.tile_pool(name="p3y", bufs=2, space="PSUM"))
                    ypool = psstack.enter_context(
                        tc.tile_pool(name="ysb", bufs=6))

                    # c_proj 3-term products, heads paired inside each
                    # DoubleRow: (ot level, head pair base, wo level).
                    # Head-pair (0,1) terms first: in the final drain they
                    # are ready before the last head's finalize completes.
                    CP_TERMS = ((0, 0, 0), (1, 0, 0), (0, 0, 1),
                                (0, 2, 0), (1, 2, 0), (0, 2, 1))

                    def cproj_gen(qc, terms=None, out_t=None, use_act_fn=None):
                        """Yield c_proj micro-ops (closures) for qc's four
                        t-blocks; each op is one DoubleRow matmul or the
                        psum->sbuf descale + store DMA of one (tb, j)
                        chain.  The double-buffered psum pool lets chain
                        i+1's matmuls run while chain i's drain completes.
                        `terms` selects a subset of CP_TERMS (used to split
                        the last qc into head-pair halves), `out_t` the
                        output tensor (with a row offset for y2)."""
                        terms = CP_TERMS if terms is None else terms
                        for tb in range(4 * qc, 4 * qc + 4):
                            for j in range(4):
                                state = {}

                                def op_mm(ti, tb=tb, j=j, state=state,
                                          terms=terms):
                                    if ti == 0:
                                        state["y"] = ps_y.tile(
                                            [P, TC], F32, tag="ya", name="ya")
                                    lo, h0, lw = terms[ti]
                                    nc.tensor.matmul(
                                        state["y"],
                                        otn_sb[:, lo, h0:h0 + 2,
                                               tb * P:(tb + 1) * P],
                                        wo_sb[:, lw, h0:h0 + 2,
                                              j * TC:(j + 1) * TC],
                                        start=(ti == 0),
                                        stop=(ti == len(terms) - 1),
                                        perf_mode=DR)

                                for ti in range(len(terms)):
                                    yield (lambda ti=ti, f=op_mm: f(ti))

                                use_act = (use_act_fn is not None
                                           and use_act_fn(tb, j))

                                def op_fin(tb=tb, j=j, state=state,
                                           use_act=use_act, out_t=out_t):
                                    y_sb = ypool.tile([P, TC], y_dt,
                                                      tag="y_sb",
                                                      name="y_sb")
                                    if use_act:
                                        nc.scalar.mul(y_sb, state["y"],
                                                      1.0 / WS)
                                    else:
                                        nc.vector.tensor_scalar_mul(
                                            out=y_sb, in0=state["y"],
                                            scalar1=invws)
                                    dst, row0 = (y, 0) if out_t is None \
                                        else out_t
                                    nc.sync.dma_start(
                                        out=dst[tb * P - row0:
                                                (tb + 1) * P - row0,
                                                j * TC:(j + 1) * TC],
                                        in_=y_sb)

                                yield op_fin

                    pending = []

                    def drain(n):
                        # emit up to n pending micro-ops
                        for _ in range(n):
                            if not pending:
                                return
                            pending.pop()()

                    # V projections for chunks 4-7 run as qc0 filler ops
                    # (qc0 has no c_proj backlog to interleave): their solid
                    # DoubleRow blocks keep the PE queue fed while qc0's exp
                    # chains (and the Sqrt->Exp act-table switch) retire.
                    # Their psums borrow the c_proj pool, idle until qc1.
                    def v_tail_ops(tc_i):
                        state = {}

                        def op_a(tc_i=tc_i, state=state):
                            ps = ps_y.tile([P, TC], F32, tag="ya",
                                           name="vd_ps")
                            state["ps"] = ps
                            for i, (wl, xl) in enumerate(L3_TERMS[:2]):
                                for kp in range(NKP):
                                    nc.tensor.matmul(
                                        ps[:, 0:TC1],
                                        wv_sb[:, wl, 2 * kp:2 * kp + 2, :],
                                        xts_tiles[tc_i][
                                            :, xl, 2 * kp:2 * kp + 2, :],
                                        start=(i == 0 and kp == 0),
                                        stop=False, perf_mode=DR)

                        def op_b(tc_i=tc_i, state=state):
                            ps = state["ps"]
                            wl, xl = L3_TERMS[2]
                            for kp in range(NKP):
                                nc.tensor.matmul(
                                    ps[:, 0:TC1],
                                    wv_sb[:, wl, 2 * kp:2 * kp + 2, :],
                                    xts_tiles[tc_i][
                                        :, xl, 2 * kp:2 * kp + 2, :],
                                    start=False, stop=(kp == NKP - 1),
                                    perf_mode=DR)
                            vt_sb = tmpool.tile([P, TC1], ST_DT, tag="vt",
                                                name="vt")
                            # drain on DVE: these run inside qc0 where the
                            # exp chain keeps ACT busy, while DVE has slack
                            nc.vector.tensor_scalar_mul(
                                out=vt_sb, in0=ps[:, 0:TC1], scalar1=invws)
                            nc.sync.dma_start_transpose(
                                v_sb[:, 2 * tc_i:2 * tc_i + 2, :], vt_sb)

                        return [op_a, op_b]

                    pending = sum((v_tail_ops(i) for i in range(4, NTC1)),
                                  [])
                    pending.reverse()

                    # rowsum bank: full-bank [P, TC] allocation keeps the
                    # interleaved accumulation chains' start-flag zero
                    # region private
                    rs_bank = ps_rs.tile([P, TC], F32, tag="rs16",
                                         name="rs16", bufs=1)

                    for qc in range(NTC):
                        qsl = slice(qc * TC, (qc + 1) * TC)
                        nkb = 4 * (qc + 1)
                        # drain rate: finish pending ops just as this qc's
                        # attention ends (fractional pacing); qc0 drains
                        # eagerly to ride out the ACT backlog from phase 1
                        rate = len(pending) / float(nkb * G)
                        if qc == 0:
                            rate *= 1.0
                        take_acc = 0.0
                        it_i = [0]
                        fin_pending = [None]
                        # rowsums for all (head, q-subblock) chains land as
                        # psum columns of the shared bank: p is the
                        # *stationary* operand and a ones-column the moving
                        # one, so each matmul costs ~1 cycle instead of TC.
                        rs16 = rs_bank
                        # zero the chain columns once, then accumulate with
                        # start=False: avoids bank-granular start-flag zeroing
                        # corrupting/serializing the 16 interleaved chains
                        nc.vector.memset(rs16[:, 0:G * 4], 0.0)
                        for h in range(G):
                            ot_ps = ps_ot.tile([P, TC], F32, tag="ot",
                                               name="ot_ps")
                            for kb in range(nkb):
                                r = kb - 4 * qc  # >=0 on diagonal blocks
                                c0 = max(r, 0) * P  # first valid q column
                                st_ps = ps_st.tile([P, TC], F32, tag="st",
                                                   name="st_ps")
                                nc.tensor.matmul(
                                    st_ps[:, c0:],
                                    kT_sb[:, kb * P:(kb + 1) * P],
                                    q_sb[h][:, qc * TC + c0:
                                            (qc + 1) * TC],
                                    start=True, stop=True)
                                # previous head's finalize chain lands here,
                                # one kb into this head's stream, so its
                                # DVE/Pool/ACT tail hides under attention
                                # matmuls instead of stalling the PE queue
                                if kb == 1 and fin_pending[0] is not None:
                                    fin_pending[0]()
                                    fin_pending[0] = None
                                # fill the PE queue *before* the exp-gated
                                # PV matmul so ACT latency is hidden.  The
                                # emission rate is back-loaded (0.5x then
                                # 1.5x): the greedy scheduler hoists any
                                # emitted-and-ready op into earlier holes,
                                # so only late emission keeps filler alive
                                # for the tail of each qc's attention.
                                it_i[0] += 1
                                take_acc += rate * (
                                    0.3 if it_i[0] * 10 < nkb * G * 7
                                    else 2.63)
                                if take_acc >= 1.0:
                                    n_take = int(take_acc)
                                    take_acc -= n_take
                                    drain(n_take)
                                p_sb = ppool.tile([P, TC], ST_DT, tag="p",
                                                  name="p_sb")
                                nc.scalar.activation(
                                    out=p_sb[:, c0:], in_=st_ps[:, c0:],
                                    func=mybir.ActivationFunctionType.Exp,
                                    scale=rk_all[:, kb:kb + 1])
                                if r >= 0:
                                    # causal mask on the diagonal strip only
                                    # (PV/rowsum read cols >= c0)
                                    nc.vector.tensor_mul(
                                        out=p_sb[:, c0:c0 + P],
                                        in0=p_sb[:, c0:c0 + P],
                                        in1=tri)
                                nc.tensor.matmul(
                                    ot_ps[:, c0:], v_sb[:, kb, :],
                                    p_sb[:, c0:], start=(kb == 0),
                                    stop=(kb == nkb - 1))
                                for sub in range(max(r, 0), 4):
                                    nc.tensor.matmul(
                                        rs16[:, h * 4 + sub:h * 4 + sub + 1],
                                        p_sb[:, sub * P:(sub + 1) * P],
                                        ones_col,
                                        start=False,
                                        stop=(kb == 4 * qc + sub),
                                        skip_group_check=True)
                            # finalize head h (deferred into head h+1's kb
                            # stream; the last head finalizes at qc end):
                            # rowsum columns -> [4, 128] rows via PE
                            # transposes, then reciprocal + per-row
                            # broadcast + norm, then the hi/lo fp8 split of
                            # OT for c_proj.
                            def finalize(h=h, qsl=qsl, ot_ps=ot_ps,
                                         rs16=rs16, split=1):
                                rs4_sb = stri2.tile([P, 4], ST_DT,
                                                    tag="rs4sb",
                                                    name="rs4_sb", bufs=2)
                                nc.vector.tensor_copy(
                                    out=rs4_sb, in_=rs16[:, h * 4:h * 4 + 4])
                                # 4 single-column bf16 transposes land every
                                # rowsum row at partition 0 of one [1, TC]
                                # psum row, so the broadcast is base-0 legal
                                rs_row = ps_rw.tile([1, TC], ST_DT,
                                                    tag="rsrow",
                                                    name="rs_row")
                                for sub in range(4):
                                    nc.tensor.matmul(
                                        rs_row[0:1, sub * P:(sub + 1) * P],
                                        rs4_sb[:, sub:sub + 1], ident,
                                        is_transpose=True,
                                        skip_group_check=True)
                                recip_row = stri2.tile([1, TC], ST_DT,
                                                       tag="reciprow",
                                                       name="recip_row",
                                                       bufs=2)
                                recipB = apool.tile([P, TC], ST_DT,
                                                    tag="recipB",
                                                    name="recipB", bufs=4)
                                otf = apool.tile([P, TC], ST_DT, tag="otf",
                                                 name="otf", bufs=2)
                                # `split` > 1 pipelines the chain in
                                # column halves across DVE/Pool/ACT --
                                # used for the very last finalize, whose
                                # latency gates the final c_proj drain.
                                w = TC // split
                                for s_ in range(split):
                                    cs = slice(s_ * w, (s_ + 1) * w)
                                    qs = slice(qsl.start + s_ * w,
                                               qsl.start + (s_ + 1) * w)
                                    nc.vector.reciprocal(
                                        out=recip_row[:, cs],
                                        in_=rs_row[:, cs])
                                    nc.gpsimd.partition_broadcast(
                                        recipB[:, cs], recip_row[:, cs])
                                    nc.vector.tensor_mul(
                                        out=otf[:, cs], in0=ot_ps[:, cs],
                                        in1=recipB[:, cs])
                                    nc.vector.tensor_copy(
                                        out=otn_sb[:, 0, h, qs],
                                        in_=otf[:, cs])
                                    nc.vector.tensor_sub(
                                        out=otn_sb[:, 1, h, qs],
                                        in0=otf[:, cs],
                                        in1=otn_sb[:, 0, h, qs])

                            fin_pending[0] = finalize

                        # leftover ops from the previous qc, then the last
                        # head's finalize (before the next qc's rs16 memset),
                        # then queue this qc's c_proj for interleaving into
                        # the next qc's attention.
                        drain(10 ** 6)
                        fin_pending[0](split=4 if qc == NTC - 1 else 1)
                        fin_pending[0] = None
                        pending = list(cproj_gen(
                            qc,
                            use_act_fn=(lambda tb, j: (tb + j) % 2)
                            if qc == NTC - 1 else None))
                        pending.reverse()
                    drain(10 ** 6)

    nc.compile()
    return nc


_NC_CACHE = None


def _get_program():
    global _NC_CACHE
    if _NC_CACHE is None:
        _NC_CACHE = build_program()
    return _NC_CACHE


def _make_tables(pos):
    half = HD // 2
    inv_freq = 1.0 / (THETA ** (np.arange(half, dtype=np.float64) / half))
    ang = (pos + np.arange(T, dtype=np.float64))[None, :] * inv_freq[:, None]
    cos = np.cos(ang).astype(np.float32)
    sin = np.sin(ang).astype(np.float32)
    cosT = np.ascontiguousarray(np.concatenate([cos, cos], axis=0))
    sinT = np.ascontiguousarray(np.concatenate([-sin, sin], axis=0))
    return cosT, sinT


def _split8(a):
    """hi/lo fp8e4m3 decomposition: a ~= hi + lo (elementwise)."""
    a = np.asarray(a, dtype=np.float32)
    hi = a.astype(NP_F8)
    lo = (a - hi.astype(np.float32)).astype(NP_F8)
    return hi, lo


def make_in_maps(x, Wq, Wk, Wv, Wo, q_gamma, k_gamma, pos):
    x = np.asarray(x, dtype=np.float32)
    Wq = np.asarray(Wq, dtype=np.float32)
    Wk = np.asarray(Wk, dtype=np.float32)
    Wv = np.asarray(Wv, dtype=np.float32)
    Wo = np.asarray(Wo, dtype=np.float32)
    q_gamma = np.asarray(q_gamma, dtype=np.float32)
    k_gamma = np.asarray(k_gamma, dtype=np.float32)
    pos = int(np.asarray(pos))

    cosT, sinT = _make_tables(pos)
    tabs = np.ascontiguousarray(np.stack([cosT, sinT], axis=1)
                                .astype(NP_ST))
    gamma2 = np.ascontiguousarray((q_gamma * k_gamma).reshape(P, 1)
                                  .astype(np.float32))

    # x: per batch (D, T) -> hi/lo fp8 -> [P, NTC1, 2, NKT, TC1]
    x8 = []
    for b in range(B):
        hi, lo = _split8(x[b].T)
        st = np.stack([hi.reshape(NKT, P, NTC1, TC1),
                       lo.reshape(NKT, P, NTC1, TC1)], axis=0)
        x8.append(np.ascontiguousarray(st.transpose(2, 3, 0, 1, 4)))

    # weights pre-scaled by WS, split hi/lo, packed partition-major
    qh, ql = _split8(Wq.reshape(NKT, P, NKV, G, HD) * WS)
    kh, kl = _split8(Wk.reshape(NKT, P, NKV, HD) * WS)
    vh, vl = _split8(Wv.reshape(NKT, P, NKV, HD) * WS)
    oh, ol = _split8(Wo.reshape(NKV, G, P, D) * WS)

    in_maps = []
    for c in range(N_CORES):
        b, n = divmod(c, NKV)
        wq_p = np.stack([qh[:, :, n], ql[:, :, n]], axis=0)  # (2,NKT,P,G,HD)
        wk_p = np.stack([kh[:, :, n], kl[:, :, n]], axis=0)  # (2,NKT,P,HD)
        wv_p = np.stack([vh[:, :, n], vl[:, :, n]], axis=0)
        wo_p = np.stack([oh[n], ol[n]], axis=0)              # (2,G,P,D)
        in_maps.append({
            "xT8": x8[b],
            "wq8": np.ascontiguousarray(wq_p.transpose(2, 0, 3, 1, 4)),
            "wk8": np.ascontiguousarray(wk_p.transpose(2, 0, 1, 3)),
            "wv8": np.ascontiguousarray(wv_p.transpose(2, 0, 1, 3)),
            "wo8": np.ascontiguousarray(wo_p.transpose(2, 0, 1, 3)),
            "tabs": tabs,
            "gamma2": gamma2,
        })
    return in_maps


def kernel(x, Wq, Wk, Wv, Wo, q_gamma, k_gamma, pos):
    in_maps = make_in_maps(x, Wq, Wk, Wv, Wo, q_gamma, k_gamma, pos)
    nc = _get_program()
    res = bass_utils.run_bass_kernel_spmd(nc, in_maps,
                                          core_ids=list(range(N_CORES)))
    out = np.zeros((B, T, D), dtype=np.float32)
    for c in range(N_CORES):
        b = c // NKV
        out[b] += np.asarray(res.results[c]["y"], dtype=np.float32)
    return out


if __name__ == "__main__":
    build_program()
    print("program built OK")


# revision 106
# speedup vs baseline: 1.0049x; 1.0049x over previous
"""Trainium2 Bass kernel for nn_CausalSelfAttention_35931696398729.

Sharding: 8 cores = (batch b in {0,1}) x (kv-head n in {0..3}).
Each core computes its 4 query heads' causal GQA attention for its batch
plus the partial c_proj (rows of Wo for its heads); the host sums the 4
partials per batch.  No device collectives.

Layouts are "transposed" throughout: qT/kT (d on partitions, t free) so
scores come out as ST (keys on partitions, queries free) and PV consumes
exp(ST) directly;  c_proj consumes the attention output OT (d, t) as the
stationary operand with no transposes anywhere except V (16 PE-transposes).

The projections and c_proj run as fp8e4m3 DoubleRow matmuls on hi/lo
split operands (A ~= A_hi + A_lo, both fp8): the product uses the three
dominant terms Ah*Bh + Al*Bh + Ah*Bl, which is ~11 mantissa bits of
effective precision (better than bf16) at 0.75x the bf16 PE cost, since
each DoubleRow instruction contracts two 128-deep groups at 0.5
cycles/row.  Weights are pre-scaled by WS=64 on the host so their hi/lo
parts stay in e4m3's normal range; the 1/WS descale rides the psum
drains.  Attention (QK, exp, PV) stays bf16.

QK RMSNorm is folded in without normalizing q/k tensors elementwise:
 - q-side factor r_q(t)/sqrt(HD) multiplies qT columns (query temperature)
 - k-side factor r_k(s) rides the Exp activation's per-partition scale
 - gamma_q*gamma_k multiplies kT rows (per-partition)
 - softmax runs without max-subtraction (|scores| <= sqrt(HD) after norm)
 - 1/rowsum is applied to OT columns after PV.
"""

import sys
from contextlib import ExitStack

sys.path.insert(0, "/opt/trn_rl_repo")

import ml_dtypes
import numpy as np

import concourse.bacc as bacc
import concourse.mybir as mybir
import concourse.tile as tile
from concourse import bass_utils
from concourse.masks import make_identity

B, T, D = 2, 2048, 2048
NH, NKV, HD = 16, 4, 128
G = NH // NKV  # query heads per core
EPS = 1e-6
THETA = 10000.0
N_CORES = 8
P = 128
TC = 512            # q-chunk for attention / c_proj column chunk
NTC = T // TC       # 4
TC1 = 256           # t-chunk for phase-1 projections
NTC1 = T // TC1     # 8
NKT = D // P        # 16 contraction chunks
NKP = NKT // 2      # 8 DoubleRow contraction pairs
NTB = T // P        # 16 t-blocks

WS = 64.0           # host-side weight prescale for fp8 ranges
F32 = mybir.dt.float32
F8 = mybir.dt.float8e4
ST_DT = mybir.dt.bfloat16
NP_ST = ml_dtypes.bfloat16
NP_F8 = ml_dtypes.float8_e4m3
DR = mybir.MatmulPerfMode.DoubleRow

# (w_level, x_level) term order for the 3-term hi/lo product; the w_lo
# pass runs last so each weight tensor's lo half may arrive after its hi.
L3_TERMS = ((0, 0), (0, 1), (1, 0))


def build_program():
    nc = bacc.Bacc("TRN2", target_bir_lowering=False, debug=False,
                   enable_asserts=False, num_devices=N_CORES)

    y_dt = ST_DT
    # inputs arrive host-prepacked in SBUF layout (partition-major), with
    # fp8 hi/lo splits precomputed on the host.
    xT8 = nc.dram_tensor("xT8", (P, NTC1, 2, NKT, TC1), F8,
                         kind="ExternalInput").ap()
    wq8 = nc.dram_tensor("wq8", (P, 2, G, NKT, HD), F8,
                         kind="ExternalInput").ap()
    wk8 = nc.dram_tensor("wk8", (P, 2, NKT, HD), F8,
                         kind="ExternalInput").ap()
    wv8 = nc.dram_tensor("wv8", (P, 2, NKT, HD), F8,
                         kind="ExternalInput").ap()
    wo8 = nc.dram_tensor("wo8", (P, 2, G, D), F8,
                         kind="ExternalInput").ap()
    tabs = nc.dram_tensor("tabs", (P, 2, T), ST_DT,
                          kind="ExternalInput").ap()
    gamma2 = nc.dram_tensor("gamma2", (P, 1), F32, kind="ExternalInput").ap()
    y = nc.dram_tensor("y", (T, D), y_dt, kind="ExternalOutput").ap()

    with tile.TileContext(nc) as tc, \
         nc.allow_low_precision(reason="fp8/bf16 matmul operand tiles"):
        with tc.tile_pool(name="persist", bufs=1) as persist, \
             tc.tile_pool(name="stri2", bufs=2) as stri2, \
             tc.tile_pool(name="weights", bufs=1) as wpool, \
             tc.tile_pool(name="xts", bufs=4) as xpool, \
             tc.tile_pool(name="p1tmp", bufs=3) as tmpool:
            tab_sb = persist.tile([P, 2, T], ST_DT)
            cos_sb = tab_sb[:, 0, :]
            sin_sb = tab_sb[:, 1, :]
            g2_sb = persist.tile([P, 1], F32)
            ident_f32 = persist.tile([P, P], F32)
            make_identity(nc, ident_f32)
            ident = persist.tile([P, P], ST_DT)
            nc.vector.tensor_copy(out=ident, in_=ident_f32)
            ones_f32 = persist.tile([P, P], F32)
            nc.vector.memset(ones_f32, 1.0)
            ones_col = persist.tile([P, 1], ST_DT)
            nc.vector.tensor_copy(out=ones_col, in_=ones_f32[:, 0:1])
            invws = persist.tile([P, 1], F32)
            nc.vector.memset(invws, 1.0 / WS)
            eps_k = persist.tile([P, 1], F32)
            nc.vector.memset(eps_k, EPS)
            eps_q = persist.tile([1, 1], F32)
            nc.vector.memset(eps_q, HD * EPS)
            # causal triangle mask (keep iff col >= row), built once; the
            # per-diagonal-block masking is then a cheap 2x-mode DVE multiply
            # instead of a Pool affine_select in the exp->PV chain
            tri = persist.tile([P, P], ST_DT)
            nc.vector.memset(tri, 1.0)
            nc.gpsimd.affine_select(out=tri, in_=tri, pattern=[[1, P]],
                                    compare_op=mybir.AluOpType.is_ge,
                                    fill=0.0, base=0, channel_multiplier=-1)

            q_sb = [persist.tile([P, T], ST_DT, tag=f"q_sb{h}", name=f"q_sb{h}")
                    for h in range(G)]
            kT_sb = persist.tile([P, T], ST_DT)
            v_sb = persist.tile([P, NTB, P], ST_DT)
            rk_tiles = persist.tile([P, NTB], F32)

            # ---------------- Phase 1: projections + RoPE + norms -----------
            xts_tiles = {}
            sqk_tiles = {}
            with tc.tile_pool(name="p1ps", bufs=4, space="PSUM") as ps_a, \
                 tc.tile_pool(name="p1psv", bufs=1, space="PSUM") as ps_v, \
                 tc.tile_pool(name="p1str", bufs=1, space="PSUM") as ps_s:
                wq_sb = wpool.tile([P, 2, G, NKT, HD], F8)
                wk_sb = wpool.tile([P, 2, NKT, HD], F8)
                wv_sb = wpool.tile([P, 2, NKT, HD], F8)

                def proj_mms(ps, w_sb, xts):
                    """24 DoubleRow matmuls: 3-term hi/lo product into ps."""
                    first = True
                    for wl, xl in L3_TERMS:
                        for kp in range(NKP):
                            nc.tensor.matmul(
                                ps, w_sb[:, wl, 2 * kp:2 * kp + 2, :],
                                xts[:, xl, 2 * kp:2 * kp + 2, :],
                                start=first,
                                stop=(wl, xl) == L3_TERMS[-1]
                                and kp == NKP - 1,
                                perf_mode=DR)
                            first = False

                def rk_block(ci):
                    # rk column tiles for chunk ci's key blocks:
                    # 1/sqrt(colsum(sq_k)/HD + eps); runs a full chunk after
                    # the square so the PE queue never stalls on it (and so
                    # the tabs DMA may arrive as late as chunk 1)
                    sq_k = sqk_tiles[ci]
                    for i in range(TC1 // P):
                        kb = ci * (TC1 // P) + i
                        ssqc = ps_s.tile([P, 1], F32, tag="ssqc",
                                         name="ssqc")
                        nc.tensor.matmul(ssqc, sq_k[:, i * P:(i + 1) * P],
                                         ones_f32[:, 0:1],
                                         start=True, stop=True)
                        nc.scalar.activation(
                            out=rk_tiles[:, kb:kb + 1], in_=ssqc,
                            func=mybir.ActivationFunctionType.Sqrt,
                            bias=eps_k[:], scale=float(1.0 / HD))
                        nc.vector.reciprocal(out=rk_tiles[:, kb:kb + 1],
                                             in_=rk_tiles[:, kb:kb + 1])

                def load_x(i):
                    xts = xpool.tile([P, 2, NKT, TC1], F8, tag="xts",
                                     name="xts")
                    xts_tiles[i] = xts
                    nc.sync.dma_start(out=xts, in_=xT8[:, i])

                # startup staging, ordered by when PE consumes each transfer
                # (HWDGE serializes dma_starts at ~625ns each): K weights and
                # x0 hi halves first so the first DoubleRow pass can start,
                # then x0 lo (pass 2), wk lo (pass 3), chunk-0 rope tables,
                # per-head q weights (hi then lo, matching the pass order),
                # v weights, then x and table chunks just-in-time.
                xts0 = xpool.tile([P, 2, NKT, TC1], F8, tag="xts",
                                  name="xts")
                xts_tiles[0] = xts0
                # the very first DoubleRow needs only the kt 0-1 slices of
                # wk-hi and x0-hi: land those in two tiny transfers so PE
                # starts ~1us earlier, then stream the rest
                nc.sync.dma_start(out=wk_sb[:, 0, 0:2, :],
                                  in_=wk8[:, 0, 0:2, :])
                nc.sync.dma_start(out=xts0[:, 0, 0:2, :],
                                  in_=xT8[:, 0, 0, 0:2, :])
                nc.sync.dma_start(out=wk_sb[:, 0, 2:, :],
                                  in_=wk8[:, 0, 2:, :])
                nc.sync.dma_start(out=xts0[:, 0, 2:NKP, :],
                                  in_=xT8[:, 0, 0, 2:NKP, :])
                nc.sync.dma_start(out=xts0[:, 0, NKP:, :],
                                  in_=xT8[:, 0, 0, NKP:, :])
                nc.sync.dma_start(out=xts0[:, 1], in_=xT8[:, 0, 1])
                nc.sync.dma_start(out=wk_sb[:, 1], in_=wk8[:, 1])
                for h in range(G):
                    nc.sync.dma_start(out=wq_sb[:, 0, h], in_=wq8[:, 0, h])
                    nc.sync.dma_start(out=wq_sb[:, 1, h], in_=wq8[:, 1, h])
                    if h == 0:
                        # chunk-0 rope tables: not consumed until the first
                        # rope DVE (~10us), so they ride behind head 0's
                        # weights instead of delaying them
                        nc.sync.dma_start(out=tab_sb[:, :, 0:TC1],
                                          in_=tabs[:, :, 0:TC1])
                        nc.sync.dma_start(out=g2_sb, in_=gamma2)
                nc.sync.dma_start(out=wv_sb, in_=wv8)
                load_x(1)
                nc.sync.dma_start(out=tab_sb[:, :, TC1:2 * TC1],
                                  in_=tabs[:, :, TC1:2 * TC1])
                load_x(2)
                load_x(3)
                nc.sync.dma_start(out=tab_sb[:, :, 2 * TC1:4 * TC1],
                                  in_=tabs[:, :, 2 * TC1:4 * TC1])
                nc.sync.dma_start(out=tab_sb[:, :, 4 * TC1:],
                                  in_=tabs[:, :, 4 * TC1:])

                qnorm_tails = {}
                sq_lists = {}
                rq_lists = {}

                def ssq_mm(ci, h):
                    # PE column-sum for chunk ci head h, then sqrt/recip
                    # immediately (ACT / DVE are free here); runs a chunk
                    # after the square so the PE queue never stalls on it
                    ssq = ps_s.tile([1, TC1], F32, tag="ssq",
                                    name="ssq_q", bufs=2)
                    nc.tensor.matmul(ssq, ones_col,
                                     sq_lists[ci][h],
                                     start=True, stop=True)
                    sq_s = stri2.tile([1, TC1], F32, tag="sqs",
                                      name="sq_sq", bufs=4)
                    nc.scalar.activation(
                        out=sq_s, in_=ssq,
                        func=mybir.ActivationFunctionType.Sqrt,
                        bias=eps_q[:], scale=1.0)
                    rq_row = stri2.tile([1, TC1], ST_DT, tag="rqrow",
                                        name="rq_row", bufs=4)
                    nc.vector.reciprocal(out=rq_row, in_=sq_s)
                    rq_lists[ci].append(rq_row)

                for tc_i in range(NTC1):
                    sl = slice(tc_i * TC1, (tc_i + 1) * TC1)
                    qnorm_tail = qnorm_tails.setdefault(tc_i, [])
                    sq_lists[tc_i] = []
                    rq_lists[tc_i] = []
                    if tc_i >= 4:
                        load_x(tc_i)
                    xts = xts_tiles[tc_i]
                    # all 6 projections of the chunk drain (via ACT, to
                    # bf16, descaled by 1/WS) into one batch tile; one DMA
                    # pair then builds a half-swapped copy so every rope DVE
                    # op below is all-SBUF bf16 (2x mode, aligned bases)
                    psb_all = tmpool.tile([P, 6, TC1], ST_DT, tag="pall",
                                          name="pall", bufs=2)

                    # ---- K ----
                    ps = ps_a.tile([P, TC1], F32, tag="proj", name="ps_k")
                    proj_mms(ps, wk_sb, xts)
                    nc.scalar.mul(psb_all[:, 0, :], ps, 1.0 / WS)
                    # RoPE is a rotation, so per-token norms are the same
                    # before and after it: the norm squares read the raw
                    # projection drain, decoupling the whole sqrt/recip
                    # chain from the serial DVE rope tail.
                    sqt = tmpool.tile([P, TC1], F32, tag="ropesq",
                                      name="ropesq")
                    nc.scalar.square(out=sqt, in_=psb_all[:, 0, :])
                    sqk_tiles[tc_i] = sqt

                    # ---- Q heads (prev chunk's norm PE bits interleave) ----
                    for h in range(G):
                        ps = ps_a.tile([P, TC1], F32, tag="proj",
                                       name="ps_q")
                        proj_mms(ps, wq_sb[:, :, h], xts)
                        nc.scalar.mul(psb_all[:, 1 + h, :], ps, 1.0 / WS)
                        sq_q = tmpool.tile([P, TC1], ST_DT, tag="qsq",
                                           name="sq_q", bufs=5)
                        nc.scalar.square(out=sq_q, in_=psb_all[:, 1 + h, :])
                        sq_lists[tc_i].append(sq_q)
                        if tc_i > 0:
                            ssq_mm(tc_i - 1, h)
                            if h == 0:
                                rk_block(tc_i - 1)

                    # ---- V (chunks 4-7 run at the phase boundary) ----
                    if tc_i < 4:
                        ps = ps_a.tile([P, TC1], F32, tag="proj",
                                       name="ps_vp")
                        proj_mms(ps, wv_sb, xts)
                        nc.scalar.mul(psb_all[:, 5, :], ps, 1.0 / WS)

                    # ---- batched half-swap ----
                    psw_all = tmpool.tile([P, 6, TC1], ST_DT, tag="pswp",
                                          name="pswp", bufs=2)
                    nc.sync.dma_start(out=psw_all[0:64, :, :],
                                      in_=psb_all[64:128, :, :])
                    nc.sync.dma_start(out=psw_all[64:128, :, :],
                                      in_=psb_all[0:64, :, :])

                    def rope_batch(dst, j):
                        tmp = tmpool.tile([P, TC1], ST_DT, tag="ropetmp",
                                          name="ropetmp")
                        nc.vector.tensor_mul(out=tmp,
                                             in0=psw_all[:, j, :],
                                             in1=sin_sb[:, sl])
                        tmp2 = tmpool.tile([P, TC1], ST_DT, tag="ropetmp2",
                                           name="ropetmp2")
                        nc.vector.tensor_mul(out=tmp2,
                                             in0=psb_all[:, j, :],
                                             in1=cos_sb[:, sl])
                        nc.vector.tensor_add(out=dst[:, sl], in0=tmp2,
                                             in1=tmp)

                    rope_batch(kT_sb, 0)
                    # gamma2 applied after the (pre-rope) norm-square
                    nc.vector.tensor_scalar_mul(out=kT_sb[:, sl],
                                                in0=kT_sb[:, sl],
                                                scalar1=g2_sb)
                    for h in range(G):
                        rope_batch(q_sb[h], 1 + h)

                        def qnorm(h=h, sl=sl, ci=tc_i):
                            # rq row -> all partitions on Pool (PE is the
                            # bottleneck; Pool idles in phase 1)
                            rb_sb = tmpool.tile([P, TC1], ST_DT, tag="rqb",
                                                name="rb_sb", bufs=2)
                            nc.gpsimd.partition_broadcast(
                                rb_sb, rq_lists[ci][h])
                            nc.vector.tensor_mul(out=q_sb[h][:, sl],
                                                 in0=q_sb[h][:, sl],
                                                 in1=rb_sb)

                        qnorm_tail.append(qnorm)

                    # PE transposes of V read the batch tile directly
                    if tc_i < 4:
                        for i in range(TC1 // P):
                            pst = ps_v.tile([P, P], ST_DT, tag="vtr",
                                            name="pst")
                            nc.tensor.transpose(
                                pst, psb_all[:, 5, i * P:(i + 1) * P],
                                ident)
                            nc.vector.tensor_copy(
                                out=v_sb[:, tc_i * (TC1 // P) + i, :],
                                in_=pst)
                    # previous chunk's temper finalizers
                    for fn_ in qnorm_tails.get(tc_i - 1, []):
                        fn_()
                    if tc_i == NTC1 - 1:
                        # chunk 7's norm tail runs here so every Sqrt
                        # activation precedes the first phase-2 Exp in the
                        # schedule (one act-table switch, not a thrash)
                        for h in range(G):
                            ssq_mm(tc_i, h)
                        rk_block(tc_i)
                        for fn_ in qnorm_tails[tc_i]:
                            fn_()

                # V projections for chunks 4-7 are deferred into phase 2 as
                # qc0 filler ops (see v_tail_ops below).

                # whole-tile copy of the exp scales: phase-2 exps read this
                # copy, so they depend on EVERY rk column (not just their
                # own kb slice) and the scheduler cannot hoist the first
                # exps in between chunk 7's Sqrt chains on the ACT queue --
                # which would thrash the activation-function table (no set
                # holds both Sqrt and Exp).
                rk_all = persist.tile([P, NTB], F32)
                nc.vector.tensor_copy(out=rk_all, in_=rk_tiles)

            # ---------------- Phase 2: attention ---------------------------
            with ExitStack() as p2stack:
                wopool = p2stack.enter_context(
                    tc.tile_pool(name="wo", bufs=1))
                apool = p2stack.enter_context(
                    tc.tile_pool(name="attn", bufs=2))
                ppool = p2stack.enter_context(
                    tc.tile_pool(name="psb", bufs=6))
                otpool = p2stack.enter_context(
                    tc.tile_pool(name="otn", bufs=1))
                wo_sb = wopool.tile([P, 2, G, D], F8)
                for h in range(G):
                    nc.sync.dma_start(out=wo_sb[:, 0, h], in_=wo8[:, 0, h])
                for h in range(G):
                    nc.sync.dma_start(out=wo_sb[:, 1, h], in_=wo8[:, 1, h])
                # hi/lo fp8 attention output, heads adjacent per level so a
                # DoubleRow lhsT can pair two heads' d-blocks
                otn_sb = otpool.tile([P, 2, G, T], F8)

                with ExitStack() as psstack:
                    ps_st = psstack.enter_context(
                        tc.tile_pool(name="p2st", bufs=2, space="PSUM"))
                    ps_ot = psstack.enter_context(
                        tc.tile_pool(name="p2ot", bufs=2, space="PSUM"))
                    ps_rs = psstack.enter_context(
                        tc.tile_pool(name="p2rs", bufs=1, space="PSUM"))
                    ps_rw = psstack.enter_context(
                        tc.tile_pool(name="p2rw", bufs=1, space="PSUM"))
                    ps_y = psstack.enter_context(
                        tc.tile_pool(name="p3y", bufs=2, space="PSUM"))
                    ypool = psstack.enter_context(
                        tc.tile_pool(name="ysb", bufs=6))

                    # c_proj 3-term products, heads paired inside each
                    # DoubleRow: (ot level, head pair base, wo level).
                    # Head-pair (0,1) terms first: in the final drain they
                    # are ready before the last head's finalize completes.
                    CP_TERMS = ((0, 0, 0), (1, 0, 0), (0, 0, 1),
                                (0, 2, 0), (1, 2, 0), (0, 2, 1))

                    def cproj_gen(qc, terms=None, out_t=None, use_act_fn=None):
                        """Yield c_proj micro-ops (closures) for qc's four
                        t-blocks; each op is one DoubleRow matmul or the
                        psum->sbuf descale + store DMA of one (tb, j)
                        chain.  The double-buffered psum pool lets chain
                        i+1's matmuls run while chain i's drain completes.
                        `terms` selects a subset of CP_TERMS (used to split
                        the last qc into head-pair halves), `out_t` the
                        output tensor (with a row offset for y2)."""
                        terms = CP_TERMS if terms is None else terms
                        for tb in range(4 * qc, 4 * qc + 4):
                            for j in range(4):
                                state = {}

                                def op_mm(ti, tb=tb, j=j, state=state,
                                          terms=terms):
                                    if ti == 0:
                                        state["y"] = ps_y.tile(
                                            [P, TC], F32, tag="ya", name="ya")
                                    lo, h0, lw = terms[ti]
                                    nc.tensor.matmul(
                                        state["y"],
                                        otn_sb[:, lo, h0:h0 + 2,
                                               tb * P:(tb + 1) * P],
                                        wo_sb[:, lw, h0:h0 + 2,
                                              j * TC:(j + 1) * TC],
                                        start=(ti == 0),
                                        stop=(ti == len(terms) - 1),
                                        perf_mode=DR)

                                for ti in range(len(terms)):
                                    yield (lambda ti=ti, f=op_mm: f(ti))

                                use_act = (use_act_fn is not None
                                           and use_act_fn(tb, j))

                                def op_fin(tb=tb, j=j, state=state,
                                           use_act=use_act, out_t=out_t):
                                    y_sb = ypool.tile([P, TC], y_dt,
                                                      tag="y_sb",
                                                      name="y_sb")
                                    if use_act:
                                        nc.scalar.mul(y_sb, state["y"],
                                                      1.0 / WS)
                                    else:
                                        nc.vector.tensor_scalar_mul(
                                            out=y_sb, in0=state["y"],
                                            scalar1=invws)
                                    dst, row0 = (y, 0) if out_t is None \
                                        else out_t
                                    nc.sync.dma_start(
                                        out=dst[tb * P - row0:
                                                (tb + 1) * P - row0,
                                                j * TC:(j + 1) * TC],
                                        in_=y_sb)

                                yield op_fin

                    pending = []

                    def drain(n):
                        # emit up to n pending micro-ops
                        for _ in range(n):
                            if not pending:
                                return
                            pending.pop()()

                    # V projections for chunks 4-7 run as qc0 filler ops
                    # (qc0 has no c_proj backlog to interleave): their solid
                    # DoubleRow blocks keep the PE queue fed while qc0's exp
                    # chains (and the Sqrt->Exp act-table switch) retire.
                    # Their psums borrow the c_proj pool, idle until qc1.
                    def v_tail_ops(tc_i):
                        state = {}

                        def op_a(tc_i=tc_i, state=state):
                            ps = ps_y.tile([P, TC], F32, tag="ya",
                                           name="vd_ps")
                            state["ps"] = ps
                            for i, (wl, xl) in enumerate(L3_TERMS[:2]):
                                for kp in range(NKP):
                                    nc.tensor.matmul(
                                        ps[:, 0:TC1],
                                        wv_sb[:, wl, 2 * kp:2 * kp + 2, :],
                                        xts_tiles[tc_i][
                                            :, xl, 2 * kp:2 * kp + 2, :],
                                        start=(i == 0 and kp == 0),
                                        stop=False, perf_mode=DR)

                        def op_b(tc_i=tc_i, state=state):
                            ps = state["ps"]
                            wl, xl = L3_TERMS[2]
                            for kp in range(NKP):
                                nc.tensor.matmul(
                                    ps[:, 0:TC1],
                                    wv_sb[:, wl, 2 * kp:2 * kp + 2, :],
                                    xts_tiles[tc_i][
                                        :, xl, 2 * kp:2 * kp + 2, :],
                                    start=False, stop=(kp == NKP - 1),
                                    perf_mode=DR)
                            vt_sb = tmpool.tile([P, TC1], ST_DT, tag="vt",
                                                name="vt")
                            # drain on DVE: these run inside qc0 where the
                            # exp chain keeps ACT busy, while DVE has slack
                            nc.vector.tensor_scalar_mul(
                                out=vt_sb, in0=ps[:, 0:TC1], scalar1=invws)
                            nc.sync.dma_start_transpose(
                                v_sb[:, 2 * tc_i:2 * tc_i + 2, :], vt_sb)

                        return [op_a, op_b]

                    pending = sum((v_tail_ops(i) for i in range(4, NTC1)),
                                  [])
                    pending.reverse()

                    # rowsum bank: full-bank [P, TC] allocation keeps the
                    # interleaved accumulation chains' start-flag zero
                    # region private
                    rs_bank = ps_rs.tile([P, TC], F32, tag="rs16",
                                         name="rs16", bufs=1)

                    for qc in range(NTC):
                        qsl = slice(qc * TC, (qc + 1) * TC)
                        nkb = 4 * (qc + 1)
                        # drain rate: finish pending ops just as this qc's
                        # attention ends (fractional pacing); qc0 drains
                        # eagerly to ride out the ACT backlog from phase 1
                        rate = len(pending) / float(nkb * G)
                        if qc == 0:
                            rate *= 1.0
                        take_acc = 0.0
                        it_i = [0]
                        fin_pending = [None]
                        # rowsums for all (head, q-subblock) chains land as
                        # psum columns of the shared bank: p is the
                        # *stationary* operand and a ones-column the moving
                        # one, so each matmul costs ~1 cycle instead of TC.
                        rs16 = rs_bank
                        # zero the chain columns once, then accumulate with
                        # start=False: avoids bank-granular start-flag zeroing
                        # corrupting/serializing the 16 interleaved chains
                        nc.vector.memset(rs16[:, 0:G * 4], 0.0)
                        for h in range(G):
                            ot_ps = ps_ot.tile([P, TC], F32, tag="ot",
                                               name="ot_ps")
                            for kb in range(nkb):
                                r = kb - 4 * qc  # >=0 on diagonal blocks
                                c0 = max(r, 0) * P  # first valid q column
                                st_ps = ps_st.tile([P, TC], F32, tag="st",
                                                   name="st_ps")
                                nc.tensor.matmul(
                                    st_ps[:, c0:],
                                    kT_sb[:, kb * P:(kb + 1) * P],
                                    q_sb[h][:, qc * TC + c0:
                                            (qc + 1) * TC],
                                    start=True, stop=True)
                                # previous head's finalize chain lands here,
                                # one kb into this head's stream, so its
                                # DVE/Pool/ACT tail hides under attention
                                # matmuls instead of stalling the PE queue
                                if kb == 1 and fin_pending[0] is not None:
                                    fin_pending[0]()
                                    fin_pending[0] = None
                                # fill the PE queue *before* the exp-gated
                                # PV matmul so ACT latency is hidden.  The
                                # emission rate is back-loaded (0.5x then
                                # 1.5x): the greedy scheduler hoists any
                                # emitted-and-ready op into earlier holes,
                                # so only late emission keeps filler alive
                                # for the tail of each qc's attention.
                                it_i[0] += 1
                                take_acc += rate * (
                                    0.3 if it_i[0] * 10 < nkb * G * 7
                                    else 2.63)
                                if take_acc >= 1.0:
                                    n_take = int(take_acc)
                                    take_acc -= n_take
                                    drain(n_take)
                                p_sb = ppool.tile([P, TC], ST_DT, tag="p",
                                                  name="p_sb")
                                nc.scalar.activation(
                                    out=p_sb[:, c0:], in_=st_ps[:, c0:],
                                    func=mybir.ActivationFunctionType.Exp,
                                    scale=rk_all[:, kb:kb + 1])
                                if r >= 0:
                                    # causal mask on the diagonal strip only
                                    # (PV/rowsum read cols >= c0)
                                    nc.vector.tensor_mul(
                                        out=p_sb[:, c0:c0 + P],
                                        in0=p_sb[:, c0:c0 + P],
                                        in1=tri)
                                nc.tensor.matmul(
                                    ot_ps[:, c0:], v_sb[:, kb, :],
                                    p_sb[:, c0:], start=(kb == 0),
                                    stop=(kb == nkb - 1))
                                for sub in range(max(r, 0), 4):
                                    nc.tensor.matmul(
                                        rs16[:, h * 4 + sub:h * 4 + sub + 1],
                                        p_sb[:, sub * P:(sub + 1) * P],
                                        ones_col,
                                        start=False,
                                        stop=(kb == 4 * qc + sub),
                                        skip_group_check=True)
                            # finalize head h (deferred into head h+1's kb
                            # stream; the last head finalizes at qc end):
                            # rowsum columns -> [4, 128] rows via PE
                            # transposes, then reciprocal + per-row
                            # broadcast + norm, then the hi/lo fp8 split of
                            # OT for c_proj.
                            def finalize(h=h, qsl=qsl, ot_ps=ot_ps,
                                         rs16=rs16, split=1):
                                rs4_sb = stri2.tile([P, 4], ST_DT,
                                                    tag="rs4sb",
                                                    name="rs4_sb", bufs=2)
                                nc.vector.tensor_copy(
                                    out=rs4_sb, in_=rs16[:, h * 4:h * 4 + 4])
                                # 4 single-column bf16 transposes land every
                                # rowsum row at partition 0 of one [1, TC]
                                # psum row, so the broadcast is base-0 legal
                                rs_row = ps_rw.tile([1, TC], ST_DT,
                                                    tag="rsrow",
                                                    name="rs_row")
                                for sub in range(4):
                                    nc.tensor.matmul(
                                        rs_row[0:1, sub * P:(sub + 1) * P],
                                        rs4_sb[:, sub:sub + 1], ident,
                                        is_transpose=True,
                                        skip_group_check=True)
                                recip_row = stri2.tile([1, TC], ST_DT,
                                                       tag="reciprow",
                                                       name="recip_row",
                                                       bufs=2)
                                recipB = apool.tile([P, TC], ST_DT,
                                                    tag="recipB",
                                                    name="recipB", bufs=4)
                                otf = apool.tile([P, TC], ST_DT, tag="otf",
                                                 name="otf", bufs=2)
                                # `split` > 1 pipelines the chain in
                                # column halves across DVE/Pool/ACT --
                                # used for the very last finalize, whose
                                # latency gates the final c_proj drain.
                                w = TC // split
                                for s_ in range(split):
                                    cs = slice(s_ * w, (s_ + 1) * w)
                                    qs = slice(qsl.start + s_ * w,
                                               qsl.start + (s_ + 1) * w)
                                    nc.vector.reciprocal(
                                        out=recip_row[:, cs],
                                        in_=rs_row[:, cs])
                                    nc.gpsimd.partition_broadcast(
                                        recipB[:, cs], recip_row[:, cs])
                                    nc.vector.tensor_mul(
                                        out=otf[:, cs], in0=ot_ps[:, cs],
                                        in1=recipB[:, cs])
                                    nc.vector.tensor_copy(
                                        out=otn_sb[:, 0, h, qs],
                                        in_=otf[:, cs])
                                    nc.vector.tensor_sub(
                                        out=otn_sb[:, 1, h, qs],
                                        in0=otf[:, cs],
                                        in1=otn_sb[:, 0, h, qs])

                            fin_pending[0] = finalize

                        # leftover ops from the previous qc, then the last
                        # head's finalize (before the next qc's rs16 memset),
                        # then queue this qc's c_proj for interleaving into
                        # the next qc's attention.
                        drain(10 ** 6)
                        fin_pending[0](split=4 if qc == NTC - 1 else 1)
                        fin_pending[0] = None
                        pending = list(cproj_gen(
                            qc,
                            use_act_fn=(lambda tb, j: (tb + j) % 2)
                            if qc == NTC - 1 else None))
                        pending.reverse()
                    drain(10 ** 6)

    nc.compile()
    return nc


_NC_CACHE = None


def _get_program():
    global _NC_CACHE
    if _NC_CACHE is None:
        _NC_CACHE = build_program()
    return _NC_CACHE


def _make_tables(pos):
    half = HD // 2
    inv_freq = 1.0 / (THETA ** (np.arange(half, dtype=np.float64) / half))
    ang = (pos + np.arange(T, dtype=np.float64))[None, :] * inv_freq[:, None]
    cos = np.cos(ang).astype(np.float32)
    sin = np.sin(ang).astype(np.float32)
    cosT = np.ascontiguousarray(np.concatenate([cos, cos], axis=0))
    sinT = np.ascontiguousarray(np.concatenate([-sin, sin], axis=0))
    return cosT, sinT


def _split8(a):
    """hi/lo fp8e4m3 decomposition: a ~= hi + lo (elementwise)."""
    a = np.asarray(a, dtype=np.float32)
    hi = a.astype(NP_F8)
    lo = (a - hi.astype(np.float32)).astype(NP_F8)
    return hi, lo


def make_in_maps(x, Wq, Wk, Wv, Wo, q_gamma, k_gamma, pos):
    x = np.asarray(x, dtype=np.float32)
    Wq = np.asarray(Wq, dtype=np.float32)
    Wk = np.asarray(Wk, dtype=np.float32)
    Wv = np.asarray(Wv, dtype=np.float32)
    Wo = np.asarray(Wo, dtype=np.float32)
    q_gamma = np.asarray(q_gamma, dtype=np.float32)
    k_gamma = np.asarray(k_gamma, dtype=np.float32)
    pos = int(np.asarray(pos))

    cosT, sinT = _make_tables(pos)
    tabs = np.ascontiguousarray(np.stack([cosT, sinT], axis=1)
                                .astype(NP_ST))
    gamma2 = np.ascontiguousarray((q_gamma * k_gamma).reshape(P, 1)
                                  .astype(np.float32))

    # x: per batch (D, T) -> hi/lo fp8 -> [P, NTC1, 2, NKT, TC1]
    x8 = []
    for b in range(B):
        hi, lo = _split8(x[b].T)
        st = np.stack([hi.reshape(NKT, P, NTC1, TC1),
                       lo.reshape(NKT, P, NTC1, TC1)], axis=0)
        x8.append(np.ascontiguousarray(st.transpose(2, 3, 0, 1, 4)))

    # weights pre-scaled by WS, split hi/lo, packed partition-major
    qh, ql = _split8(Wq.reshape(NKT, P, NKV, G, HD) * WS)
    kh, kl = _split8(Wk.reshape(NKT, P, NKV, HD) * WS)
    vh, vl = _split8(Wv.reshape(NKT, P, NKV, HD) * WS)
    oh, ol = _split8(Wo.reshape(NKV, G, P, D) * WS)

    in_maps = []
    for c in range(N_CORES):
        b, n = divmod(c, NKV)
        wq_p = np.stack([qh[:, :, n], ql[:, :, n]], axis=0)  # (2,NKT,P,G,HD)
        wk_p = np.stack([kh[:, :, n], kl[:, :, n]], axis=0)  # (2,NKT,P,HD)
        wv_p = np.stack([vh[:, :, n], vl[:, :, n]], axis=0)
        wo_p = np.stack([oh[n], ol[n]], axis=0)              # (2,G,P,D)
        in_maps.append({
            "xT8": x8[b],
            "wq8": np.ascontiguousarray(wq_p.transpose(2, 0, 3, 1, 4)),
            "wk8": np.ascontiguousarray(wk_p.transpose(2, 0, 1, 3)),
            "wv8": np.ascontiguousarray(wv_p.transpose(2, 0, 1, 3)),
            "wo8": np.ascontiguousarray(wo_p.transpose(2, 0, 1, 3)),
            "tabs": tabs,
            "gamma2": gamma2,
        })
    return in_maps


def kernel(x, Wq, Wk, Wv, Wo, q_gamma, k_gamma, pos):
    in_maps = make_in_maps(x, Wq, Wk, Wv, Wo, q_gamma, k_gamma, pos)
    nc = _get_program()
    res = bass_utils.run_bass_kernel_spmd(nc, in_maps,
                                          core_ids=list(range(N_CORES)))
    out = np.zeros((B, T, D), dtype=np.float32)
    for c in range(N_CORES):
        b = c // NKV
        out[b] += np.asarray(res.results[c]["y"], dtype=np.float32)
    return out


if __name__ == "__main__":
    build_program()
    print("program built OK")


# revision 107
# speedup vs baseline: 1.0129x; 1.0079x over previous
"""Trainium2 Bass kernel for nn_CausalSelfAttention_35931696398729.

Sharding: 8 cores = (batch b in {0,1}) x (kv-head n in {0..3}).
Each core computes its 4 query heads' causal GQA attention for its batch
plus the partial c_proj (rows of Wo for its heads); the host sums the 4
partials per batch.  No device collectives.

Layouts are "transposed" throughout: qT/kT (d on partitions, t free) so
scores come out as ST (keys on partitions, queries free) and PV consumes
exp(ST) directly;  c_proj consumes the attention output OT (d, t) as the
stationary operand with no transposes anywhere except V (16 PE-transposes).

The projections and c_proj run as fp8e4m3 DoubleRow matmuls on hi/lo
split operands (A ~= A_hi + A_lo, both fp8): the product uses the three
dominant terms Ah*Bh + Al*Bh + Ah*Bl, which is ~11 mantissa bits of
effective precision (better than bf16) at 0.75x the bf16 PE cost, since
each DoubleRow instruction contracts two 128-deep groups at 0.5
cycles/row.  Weights are pre-scaled by WS=64 on the host so their hi/lo
parts stay in e4m3's normal range; the 1/WS descale rides the psum
drains.  Attention (QK, exp, PV) stays bf16.

QK RMSNorm is folded in without normalizing q/k tensors elementwise:
 - q-side factor r_q(t)/sqrt(HD) multiplies qT columns (query temperature)
 - k-side factor r_k(s) rides the Exp activation's per-partition scale
 - gamma_q*gamma_k multiplies kT rows (per-partition)
 - softmax runs without max-subtraction (|scores| <= sqrt(HD) after norm)
 - 1/rowsum is applied to OT columns after PV.
"""

import sys
from contextlib import ExitStack

sys.path.insert(0, "/opt/trn_rl_repo")

import ml_dtypes
import numpy as np

import concourse.bacc as bacc
import concourse.mybir as mybir
import concourse.tile as tile
from concourse import bass_utils
from concourse.masks import make_identity

B, T, D = 2, 2048, 2048
NH, NKV, HD = 16, 4, 128
G = NH // NKV  # query heads per core
EPS = 1e-6
THETA = 10000.0
N_CORES = 8
P = 128
TC = 512            # q-chunk for attention / c_proj column chunk
NTC = T // TC       # 4
TC1 = 256           # t-chunk for phase-1 projections
NTC1 = T // TC1     # 8
NKT = D // P        # 16 contraction chunks
NKP = NKT // 2      # 8 DoubleRow contraction pairs
NTB = T // P        # 16 t-blocks

WS = 64.0           # host-side weight prescale for fp8 ranges
F32 = mybir.dt.float32
F8 = mybir.dt.float8e4
ST_DT = mybir.dt.bfloat16
NP_ST = ml_dtypes.bfloat16
NP_F8 = ml_dtypes.float8_e4m3
DR = mybir.MatmulPerfMode.DoubleRow

# (w_level, x_level) term order for the 3-term hi/lo product; the w_lo
# pass runs last so each weight tensor's lo half may arrive after its hi.
L3_TERMS = ((0, 0), (0, 1), (1, 0))


def build_program():
    nc = bacc.Bacc("TRN2", target_bir_lowering=False, debug=False,
                   enable_asserts=False, num_devices=N_CORES)

    y_dt = ST_DT
    # inputs arrive host-prepacked in SBUF layout (partition-major), with
    # fp8 hi/lo splits precomputed on the host.
    xT8 = nc.dram_tensor("xT8", (P, NTC1, 2, NKT, TC1), F8,
                         kind="ExternalInput").ap()
    wq8 = nc.dram_tensor("wq8", (P, 2, G, NKT, HD), F8,
                         kind="ExternalInput").ap()
    wk8 = nc.dram_tensor("wk8", (P, 2, NKT, HD), F8,
                         kind="ExternalInput").ap()
    wv8 = nc.dram_tensor("wv8", (P, 2, NKT, HD), F8,
                         kind="ExternalInput").ap()
    wo8 = nc.dram_tensor("wo8", (P, 2, G, D), F8,
                         kind="ExternalInput").ap()
    tabs = nc.dram_tensor("tabs", (P, 2, T), ST_DT,
                          kind="ExternalInput").ap()
    gamma2 = nc.dram_tensor("gamma2", (P, 1), F32, kind="ExternalInput").ap()
    y = nc.dram_tensor("y", (T, D), y_dt, kind="ExternalOutput").ap()

    with tile.TileContext(nc) as tc, \
         nc.allow_low_precision(reason="fp8/bf16 matmul operand tiles"):
        with tc.tile_pool(name="persist", bufs=1) as persist, \
             tc.tile_pool(name="stri2", bufs=2) as stri2, \
             tc.tile_pool(name="weights", bufs=1) as wpool, \
             tc.tile_pool(name="xts", bufs=4) as xpool, \
             tc.tile_pool(name="p1tmp", bufs=3) as tmpool:
            tab_sb = persist.tile([P, 2, T], ST_DT)
            cos_sb = tab_sb[:, 0, :]
            sin_sb = tab_sb[:, 1, :]
            g2_sb = persist.tile([P, 1], F32)
            ident_f32 = persist.tile([P, P], F32)
            make_identity(nc, ident_f32)
            ident = persist.tile([P, P], ST_DT)
            nc.vector.tensor_copy(out=ident, in_=ident_f32)
            ones_f32 = persist.tile([P, P], F32)
            nc.vector.memset(ones_f32, 1.0)
            ones_col = persist.tile([P, 1], ST_DT)
            nc.vector.tensor_copy(out=ones_col, in_=ones_f32[:, 0:1])
            invws = persist.tile([P, 1], F32)
            nc.vector.memset(invws, 1.0 / WS)
            eps_k = persist.tile([P, 1], F32)
            nc.vector.memset(eps_k, EPS)
            eps_q = persist.tile([1, 1], F32)
            nc.vector.memset(eps_q, HD * EPS)
            # causal triangle mask (keep iff col >= row), built once; the
            # per-diagonal-block masking is then a cheap 2x-mode DVE multiply
            # instead of a Pool affine_select in the exp->PV chain
            tri = persist.tile([P, P], ST_DT)
            nc.vector.memset(tri, 1.0)
            nc.gpsimd.affine_select(out=tri, in_=tri, pattern=[[1, P]],
                                    compare_op=mybir.AluOpType.is_ge,
                                    fill=0.0, base=0, channel_multiplier=-1)

            q_sb = [persist.tile([P, T], ST_DT, tag=f"q_sb{h}", name=f"q_sb{h}")
                    for h in range(G)]
            kT_sb = persist.tile([P, T], ST_DT)
            v_sb = persist.tile([P, NTB, P], ST_DT)
            rk_tiles = persist.tile([P, NTB], F32)

            # ---------------- Phase 1: projections + RoPE + norms -----------
            xts_tiles = {}
            sqk_tiles = {}
            with tc.tile_pool(name="p1ps", bufs=5, space="PSUM") as ps_a, \
                 tc.tile_pool(name="p1psv", bufs=1, space="PSUM") as ps_v, \
                 tc.tile_pool(name="p1str", bufs=1, space="PSUM") as ps_s:
                wq_sb = wpool.tile([P, 2, G, NKT, HD], F8)
                wk_sb = wpool.tile([P, 2, NKT, HD], F8)
                wv_sb = wpool.tile([P, 2, NKT, HD], F8)

                def proj_mms(ps, w_sb, xts):
                    """24 DoubleRow matmuls: 3-term hi/lo product into ps."""
                    first = True
                    for wl, xl in L3_TERMS:
                        for kp in range(NKP):
                            nc.tensor.matmul(
                                ps, w_sb[:, wl, 2 * kp:2 * kp + 2, :],
                                xts[:, xl, 2 * kp:2 * kp + 2, :],
                                start=first,
                                stop=(wl, xl) == L3_TERMS[-1]
                                and kp == NKP - 1,
                                perf_mode=DR)
                            first = False

                def rk_block(ci):
                    # rk column tiles for chunk ci's key blocks:
                    # 1/sqrt(colsum(sq_k)/HD + eps); runs a full chunk after
                    # the square so the PE queue never stalls on it (and so
                    # the tabs DMA may arrive as late as chunk 1)
                    sq_k = sqk_tiles[ci]
                    for i in range(TC1 // P):
                        kb = ci * (TC1 // P) + i
                        ssqc = ps_s.tile([P, 1], F32, tag="ssqc",
                                         name="ssqc")
                        nc.tensor.matmul(ssqc, sq_k[:, i * P:(i + 1) * P],
                                         ones_f32[:, 0:1],
                                         start=True, stop=True)
                        nc.scalar.activation(
                            out=rk_tiles[:, kb:kb + 1], in_=ssqc,
                            func=mybir.ActivationFunctionType.Sqrt,
                            bias=eps_k[:], scale=float(1.0 / HD))
                        nc.vector.reciprocal(out=rk_tiles[:, kb:kb + 1],
                                             in_=rk_tiles[:, kb:kb + 1])

                def load_x(i):
                    xts = xpool.tile([P, 2, NKT, TC1], F8, tag="xts",
                                     name="xts")
                    xts_tiles[i] = xts
                    nc.sync.dma_start(out=xts, in_=xT8[:, i])

                # startup staging, ordered by when PE consumes each transfer
                # (HWDGE serializes dma_starts at ~625ns each): K weights and
                # x0 hi halves first so the first DoubleRow pass can start,
                # then x0 lo (pass 2), wk lo (pass 3), chunk-0 rope tables,
                # per-head q weights (hi then lo, matching the pass order),
                # v weights, then x and table chunks just-in-time.
                xts0 = xpool.tile([P, 2, NKT, TC1], F8, tag="xts",
                                  name="xts")
                xts_tiles[0] = xts0
                # the very first DoubleRow needs only the kt 0-1 slices of
                # wk-hi and x0-hi: land those in two tiny transfers so PE
                # starts ~1us earlier, then stream the rest
                nc.sync.dma_start(out=wk_sb[:, 0, 0:2, :],
                                  in_=wk8[:, 0, 0:2, :])
                nc.sync.dma_start(out=xts0[:, 0, 0:2, :],
                                  in_=xT8[:, 0, 0, 0:2, :])
                nc.sync.dma_start(out=wk_sb[:, 0, 2:, :],
                                  in_=wk8[:, 0, 2:, :])
                nc.sync.dma_start(out=xts0[:, 0, 2:NKP, :],
                                  in_=xT8[:, 0, 0, 2:NKP, :])
                nc.sync.dma_start(out=xts0[:, 0, NKP:, :],
                                  in_=xT8[:, 0, 0, NKP:, :])
                nc.sync.dma_start(out=xts0[:, 1], in_=xT8[:, 0, 1])
                nc.sync.dma_start(out=wk_sb[:, 1], in_=wk8[:, 1])
                for h in range(G):
                    nc.sync.dma_start(out=wq_sb[:, 0, h], in_=wq8[:, 0, h])
                    nc.sync.dma_start(out=wq_sb[:, 1, h], in_=wq8[:, 1, h])
                    if h == 0:
                        # chunk-0 rope tables: not consumed until the first
                        # rope DVE (~10us), so they ride behind head 0's
                        # weights instead of delaying them
                        nc.sync.dma_start(out=tab_sb[:, :, 0:TC1],
                                          in_=tabs[:, :, 0:TC1])
                        nc.sync.dma_start(out=g2_sb, in_=gamma2)
                nc.sync.dma_start(out=wv_sb, in_=wv8)
                load_x(1)
                nc.sync.dma_start(out=tab_sb[:, :, TC1:2 * TC1],
                                  in_=tabs[:, :, TC1:2 * TC1])
                load_x(2)
                load_x(3)
                nc.sync.dma_start(out=tab_sb[:, :, 2 * TC1:4 * TC1],
                                  in_=tabs[:, :, 2 * TC1:4 * TC1])
                nc.sync.dma_start(out=tab_sb[:, :, 4 * TC1:],
                                  in_=tabs[:, :, 4 * TC1:])

                qnorm_tails = {}
                sq_lists = {}
                rq_lists = {}

                def ssq_mm(ci, h):
                    # PE column-sum for chunk ci head h, then sqrt/recip
                    # immediately (ACT / DVE are free here); runs a chunk
                    # after the square so the PE queue never stalls on it
                    ssq = ps_s.tile([1, TC1], F32, tag="ssq",
                                    name="ssq_q", bufs=1)
                    nc.tensor.matmul(ssq, ones_col,
                                     sq_lists[ci][h],
                                     start=True, stop=True)
                    sq_s = stri2.tile([1, TC1], F32, tag="sqs",
                                      name="sq_sq", bufs=4)
                    nc.scalar.activation(
                        out=sq_s, in_=ssq,
                        func=mybir.ActivationFunctionType.Sqrt,
                        bias=eps_q[:], scale=1.0)
                    rq_row = stri2.tile([1, TC1], ST_DT, tag="rqrow",
                                        name="rq_row", bufs=4)
                    nc.vector.reciprocal(out=rq_row, in_=sq_s)
                    rq_lists[ci].append(rq_row)

                for tc_i in range(NTC1):
                    sl = slice(tc_i * TC1, (tc_i + 1) * TC1)
                    qnorm_tail = qnorm_tails.setdefault(tc_i, [])
                    sq_lists[tc_i] = []
                    rq_lists[tc_i] = []
                    if tc_i >= 4:
                        load_x(tc_i)
                    xts = xts_tiles[tc_i]
                    # all 6 projections of the chunk drain (via ACT, to
                    # bf16, descaled by 1/WS) into one batch tile; one DMA
                    # pair then builds a half-swapped copy so every rope DVE
                    # op below is all-SBUF bf16 (2x mode, aligned bases)
                    psb_all = tmpool.tile([P, 6, TC1], ST_DT, tag="pall",
                                          name="pall", bufs=2)

                    # ---- K ----
                    ps = ps_a.tile([P, TC1], F32, tag="proj", name="ps_k")
                    proj_mms(ps, wk_sb, xts)
                    nc.scalar.mul(psb_all[:, 0, :], ps, 1.0 / WS)
                    # RoPE is a rotation, so per-token norms are the same
                    # before and after it: the norm squares read the raw
                    # projection drain, decoupling the whole sqrt/recip
                    # chain from the serial DVE rope tail.
                    sqt = tmpool.tile([P, TC1], F32, tag="ropesq",
                                      name="ropesq")
                    nc.scalar.square(out=sqt, in_=psb_all[:, 0, :])
                    sqk_tiles[tc_i] = sqt

                    # ---- Q heads (prev chunk's norm PE bits interleave) ----
                    for h in range(G):
                        ps = ps_a.tile([P, TC1], F32, tag="proj",
                                       name="ps_q")
                        proj_mms(ps, wq_sb[:, :, h], xts)
                        nc.scalar.mul(psb_all[:, 1 + h, :], ps, 1.0 / WS)
                        sq_q = tmpool.tile([P, TC1], ST_DT, tag="qsq",
                                           name="sq_q", bufs=5)
                        nc.scalar.square(out=sq_q, in_=psb_all[:, 1 + h, :])
                        sq_lists[tc_i].append(sq_q)
                        if tc_i > 0:
                            ssq_mm(tc_i - 1, h)
                            if h == 0:
                                rk_block(tc_i - 1)

                    # ---- V (chunks 4-7 run at the phase boundary) ----
                    if tc_i < 4:
                        ps = ps_a.tile([P, TC1], F32, tag="proj",
                                       name="ps_vp")
                        proj_mms(ps, wv_sb, xts)
                        nc.scalar.mul(psb_all[:, 5, :], ps, 1.0 / WS)

                    # ---- batched half-swap ----
                    psw_all = tmpool.tile([P, 6, TC1], ST_DT, tag="pswp",
                                          name="pswp", bufs=2)
                    nc.sync.dma_start(out=psw_all[0:64, :, :],
                                      in_=psb_all[64:128, :, :])
                    nc.sync.dma_start(out=psw_all[64:128, :, :],
                                      in_=psb_all[0:64, :, :])

                    def rope_batch(dst, j):
                        tmp = tmpool.tile([P, TC1], ST_DT, tag="ropetmp",
                                          name="ropetmp")
                        nc.vector.tensor_mul(out=tmp,
                                             in0=psw_all[:, j, :],
                                             in1=sin_sb[:, sl])
                        tmp2 = tmpool.tile([P, TC1], ST_DT, tag="ropetmp2",
                                           name="ropetmp2")
                        nc.vector.tensor_mul(out=tmp2,
                                             in0=psb_all[:, j, :],
                                             in1=cos_sb[:, sl])
                        nc.vector.tensor_add(out=dst[:, sl], in0=tmp2,
                                             in1=tmp)

                    rope_batch(kT_sb, 0)
                    # gamma2 applied after the (pre-rope) norm-square
                    nc.vector.tensor_scalar_mul(out=kT_sb[:, sl],
                                                in0=kT_sb[:, sl],
                                                scalar1=g2_sb)
                    for h in range(G):
                        rope_batch(q_sb[h], 1 + h)

                        def qnorm(h=h, sl=sl, ci=tc_i):
                            # rq row -> all partitions on Pool (PE is the
                            # bottleneck; Pool idles in phase 1)
                            rb_sb = tmpool.tile([P, TC1], ST_DT, tag="rqb",
                                                name="rb_sb", bufs=2)
                            nc.gpsimd.partition_broadcast(
                                rb_sb, rq_lists[ci][h])
                            nc.vector.tensor_mul(out=q_sb[h][:, sl],
                                                 in0=q_sb[h][:, sl],
                                                 in1=rb_sb)

                        qnorm_tail.append(qnorm)

                    # PE transposes of V read the batch tile directly
                    if tc_i < 4:
                        for i in range(TC1 // P):
                            pst = ps_v.tile([P, P], ST_DT, tag="vtr",
                                            name="pst")
                            nc.tensor.transpose(
                                pst, psb_all[:, 5, i * P:(i + 1) * P],
                                ident)
                            nc.vector.tensor_copy(
                                out=v_sb[:, tc_i * (TC1 // P) + i, :],
                                in_=pst)
                    # previous chunk's temper finalizers
                    for fn_ in qnorm_tails.get(tc_i - 1, []):
                        fn_()
                    if tc_i == NTC1 - 1:
                        # chunk 7's norm tail runs here so every Sqrt
                        # activation precedes the first phase-2 Exp in the
                        # schedule (one act-table switch, not a thrash)
                        for h in range(G):
                            ssq_mm(tc_i, h)
                        rk_block(tc_i)
                        for fn_ in qnorm_tails[tc_i]:
                            fn_()

                # V projections for chunks 4-7 are deferred into phase 2 as
                # qc0 filler ops (see v_tail_ops below).

                # whole-tile copy of the exp scales: phase-2 exps read this
                # copy, so they depend on EVERY rk column (not just their
                # own kb slice) and the scheduler cannot hoist the first
                # exps in between chunk 7's Sqrt chains on the ACT queue --
                # which would thrash the activation-function table (no set
                # holds both Sqrt and Exp).
                rk_all = persist.tile([P, NTB], F32)
                nc.vector.tensor_copy(out=rk_all, in_=rk_tiles)

            # ---------------- Phase 2: attention ---------------------------
            with ExitStack() as p2stack:
                wopool = p2stack.enter_context(
                    tc.tile_pool(name="wo", bufs=1))
                apool = p2stack.enter_context(
                    tc.tile_pool(name="attn", bufs=2))
                ppool = p2stack.enter_context(
                    tc.tile_pool(name="psb", bufs=6))
                otpool = p2stack.enter_context(
                    tc.tile_pool(name="otn", bufs=1))
                wo_sb = wopool.tile([P, 2, G, D], F8)
                for h in range(G):
                    nc.sync.dma_start(out=wo_sb[:, 0, h], in_=wo8[:, 0, h])
                for h in range(G):
                    nc.sync.dma_start(out=wo_sb[:, 1, h], in_=wo8[:, 1, h])
                # hi/lo fp8 attention output, heads adjacent per level so a
                # DoubleRow lhsT can pair two heads' d-blocks
                otn_sb = otpool.tile([P, 2, G, T], F8)

                with ExitStack() as psstack:
                    ps_st = psstack.enter_context(
                        tc.tile_pool(name="p2st", bufs=2, space="PSUM"))
                    ps_ot = psstack.enter_context(
                        tc.tile_pool(name="p2ot", bufs=2, space="PSUM"))
                    ps_rs = psstack.enter_context(
                        tc.tile_pool(name="p2rs", bufs=1, space="PSUM"))
                    ps_rw = psstack.enter_context(
                        tc.tile_pool(name="p2rw", bufs=1, space="PSUM"))
                    ps_y = psstack.enter_context(
                        tc.tile_pool(name="p3y", bufs=2, space="PSUM"))
                    ypool = psstack.enter_context(
                        tc.tile_pool(name="ysb", bufs=6))

                    # c_proj 3-term products, heads paired inside each
                    # DoubleRow: (ot level, head pair base, wo level).
                    # Head-pair (0,1) terms first: in the final drain they
                    # are ready before the last head's finalize completes.
                    CP_TERMS = ((0, 0, 0), (1, 0, 0), (0, 0, 1),
                                (0, 2, 0), (1, 2, 0), (0, 2, 1))

                    def cproj_gen(qc, terms=None, out_t=None, use_act_fn=None):
                        """Yield c_proj micro-ops (closures) for qc's four
                        t-blocks; each op is one DoubleRow matmul or the
                        psum->sbuf descale + store DMA of one (tb, j)
                        chain.  The double-buffered psum pool lets chain
                        i+1's matmuls run while chain i's drain completes.
                        `terms` selects a subset of CP_TERMS (used to split
                        the last qc into head-pair halves), `out_t` the
                        output tensor (with a row offset for y2)."""
                        terms = CP_TERMS if terms is None else terms
                        for tb in range(4 * qc, 4 * qc + 4):
                            for j in range(4):
                                state = {}

                                def op_mm(ti, tb=tb, j=j, state=state,
                                          terms=terms):
                                    if ti == 0:
                                        state["y"] = ps_y.tile(
                                            [P, TC], F32, tag="ya", name="ya")
                                    lo, h0, lw = terms[ti]
                                    nc.tensor.matmul(
                                        state["y"],
                                        otn_sb[:, lo, h0:h0 + 2,
                                               tb * P:(tb + 1) * P],
                                        wo_sb[:, lw, h0:h0 + 2,
                                              j * TC:(j + 1) * TC],
                                        start=(ti == 0),
                                        stop=(ti == len(terms) - 1),
                                        perf_mode=DR)

                                for ti in range(len(terms)):
                                    yield (lambda ti=ti, f=op_mm: f(ti))

                                use_act = (use_act_fn is not None
                                           and use_act_fn(tb, j))

                                def op_fin(tb=tb, j=j, state=state,
                                           use_act=use_act, out_t=out_t):
                                    y_sb = ypool.tile([P, TC], y_dt,
                                                      tag="y_sb",
                                                      name="y_sb")
                                    if use_act:
                                        nc.scalar.mul(y_sb, state["y"],
                                                      1.0 / WS)
                                    else:
                                        nc.vector.tensor_scalar_mul(
                                            out=y_sb, in0=state["y"],
                                            scalar1=invws)
                                    dst, row0 = (y, 0) if out_t is None \
                                        else out_t
                                    nc.sync.dma_start(
                                        out=dst[tb * P - row0:
                                                (tb + 1) * P - row0,
                                                j * TC:(j + 1) * TC],
                                        in_=y_sb)

                                yield op_fin

                    pending = []

                    def drain(n):
                        # emit up to n pending micro-ops
                        for _ in range(n):
                            if not pending:
                                return
                            pending.pop()()

                    # V projections for chunks 4-7 run as qc0 filler ops
                    # (qc0 has no c_proj backlog to interleave): their solid
                    # DoubleRow blocks keep the PE queue fed while qc0's exp
                    # chains (and the Sqrt->Exp act-table switch) retire.
                    # Their psums borrow the c_proj pool, idle until qc1.
                    def v_tail_ops(tc_i):
                        state = {}

                        def op_a(tc_i=tc_i, state=state):
                            ps = ps_y.tile([P, TC], F32, tag="ya",
                                           name="vd_ps")
                            state["ps"] = ps
                            for i, (wl, xl) in enumerate(L3_TERMS[:2]):
                                for kp in range(NKP):
                                    nc.tensor.matmul(
                                        ps[:, 0:TC1],
                                        wv_sb[:, wl, 2 * kp:2 * kp + 2, :],
                                        xts_tiles[tc_i][
                                            :, xl, 2 * kp:2 * kp + 2, :],
                                        start=(i == 0 and kp == 0),
                                        stop=False, perf_mode=DR)

                        def op_b(tc_i=tc_i, state=state):
                            ps = state["ps"]
                            wl, xl = L3_TERMS[2]
                            for kp in range(NKP):
                                nc.tensor.matmul(
                                    ps[:, 0:TC1],
                                    wv_sb[:, wl, 2 * kp:2 * kp + 2, :],
                                    xts_tiles[tc_i][
                                        :, xl, 2 * kp:2 * kp + 2, :],
                                    start=False, stop=(kp == NKP - 1),
                                    perf_mode=DR)
                            vt_sb = tmpool.tile([P, TC1], ST_DT, tag="vt",
                                                name="vt")
                            # drain on DVE: these run inside qc0 where the
                            # exp chain keeps ACT busy, while DVE has slack
                            nc.vector.tensor_scalar_mul(
                                out=vt_sb, in0=ps[:, 0:TC1], scalar1=invws)
                            nc.sync.dma_start_transpose(
                                v_sb[:, 2 * tc_i:2 * tc_i + 2, :], vt_sb)

                        return [op_a, op_b]

                    pending = sum((v_tail_ops(i) for i in range(4, NTC1)),
                                  [])
                    pending.reverse()

                    # rowsum bank: full-bank [P, TC] allocation keeps the
                    # interleaved accumulation chains' start-flag zero
                    # region private
                    rs_bank = ps_rs.tile([P, TC], F32, tag="rs16",
                                         name="rs16", bufs=1)

                    for qc in range(NTC):
                        qsl = slice(qc * TC, (qc + 1) * TC)
                        nkb = 4 * (qc + 1)
                        # drain rate: finish pending ops just as this qc's
                        # attention ends (fractional pacing); qc0 drains
                        # eagerly to ride out the ACT backlog from phase 1
                        rate = len(pending) / float(nkb * G)
                        if qc == 0:
                            rate *= 1.0
                        take_acc = 0.0
                        it_i = [0]
                        fin_pending = [None]
                        # rowsums for all (head, q-subblock) chains land as
                        # psum columns of the shared bank: p is the
                        # *stationary* operand and a ones-column the moving
                        # one, so each matmul costs ~1 cycle instead of TC.
                        rs16 = rs_bank
                        # zero the chain columns once, then accumulate with
                        # start=False: avoids bank-granular start-flag zeroing
                        # corrupting/serializing the 16 interleaved chains
                        nc.vector.memset(rs16[:, 0:G * 4], 0.0)
                        for h in range(G):
                            ot_ps = ps_ot.tile([P, TC], F32, tag="ot",
                                               name="ot_ps")
                            for kb in range(nkb):
                                r = kb - 4 * qc  # >=0 on diagonal blocks
                                c0 = max(r, 0) * P  # first valid q column
                                st_ps = ps_st.tile([P, TC], F32, tag="st",
                                                   name="st_ps")
                                nc.tensor.matmul(
                                    st_ps[:, c0:],
                                    kT_sb[:, kb * P:(kb + 1) * P],
                                    q_sb[h][:, qc * TC + c0:
                                            (qc + 1) * TC],
                                    start=True, stop=True)
                                # previous head's finalize chain lands here,
                                # one kb into this head's stream, so its
                                # DVE/Pool/ACT tail hides under attention
                                # matmuls instead of stalling the PE queue
                                if kb == 1 and fin_pending[0] is not None:
                                    fin_pending[0]()
                                    fin_pending[0] = None
                                # fill the PE queue *before* the exp-gated
                                # PV matmul so ACT latency is hidden.  The
                                # emission rate is back-loaded (0.5x then
                                # 1.5x): the greedy scheduler hoists any
                                # emitted-and-ready op into earlier holes,
                                # so only late emission keeps filler alive
                                # for the tail of each qc's attention.
                                it_i[0] += 1
                                take_acc += rate * (
                                    0.3 if it_i[0] * 10 < nkb * G * 7
                                    else 2.63)
                                if take_acc >= 1.0:
                                    n_take = int(take_acc)
                                    take_acc -= n_take
                                    drain(n_take)
                                p_sb = ppool.tile([P, TC], ST_DT, tag="p",
                                                  name="p_sb")
                                nc.scalar.activation(
                                    out=p_sb[:, c0:], in_=st_ps[:, c0:],
                                    func=mybir.ActivationFunctionType.Exp,
                                    scale=rk_all[:, kb:kb + 1])
                                if r >= 0:
                                    # causal mask on the diagonal strip only
                                    # (PV/rowsum read cols >= c0)
                                    nc.vector.tensor_mul(
                                        out=p_sb[:, c0:c0 + P],
                                        in0=p_sb[:, c0:c0 + P],
                                        in1=tri)
                                nc.tensor.matmul(
                                    ot_ps[:, c0:], v_sb[:, kb, :],
                                    p_sb[:, c0:], start=(kb == 0),
                                    stop=(kb == nkb - 1))
                                for sub in range(max(r, 0), 4):
                                    nc.tensor.matmul(
                                        rs16[:, h * 4 + sub:h * 4 + sub + 1],
                                        p_sb[:, sub * P:(sub + 1) * P],
                                        ones_col,
                                        start=False,
                                        stop=(kb == 4 * qc + sub),
                                        skip_group_check=True)
                            # finalize head h (deferred into head h+1's kb
                            # stream; the last head finalizes at qc end):
                            # rowsum columns -> [4, 128] rows via PE
                            # transposes, then reciprocal + per-row
                            # broadcast + norm, then the hi/lo fp8 split of
                            # OT for c_proj.
                            def finalize(h=h, qsl=qsl, ot_ps=ot_ps,
                                         rs16=rs16, split=1):
                                rs4_sb = stri2.tile([P, 4], ST_DT,
                                                    tag="rs4sb",
                                                    name="rs4_sb", bufs=2)
                                nc.vector.tensor_copy(
                                    out=rs4_sb, in_=rs16[:, h * 4:h * 4 + 4])
                                # 4 single-column bf16 transposes land every
                                # rowsum row at partition 0 of one [1, TC]
                                # psum row, so the broadcast is base-0 legal
                                rs_row = ps_rw.tile([1, TC], ST_DT,
                                                    tag="rsrow",
                                                    name="rs_row")
                                for sub in range(4):
                                    nc.tensor.matmul(
                                        rs_row[0:1, sub * P:(sub + 1) * P],
                                        rs4_sb[:, sub:sub + 1], ident,
                                        is_transpose=True,
                                        skip_group_check=True)
                                recip_row = stri2.tile([1, TC], ST_DT,
                                                       tag="reciprow",
                                                       name="recip_row",
                                                       bufs=2)
                                recipB = apool.tile([P, TC], ST_DT,
                                                    tag="recipB",
                                                    name="recipB", bufs=4)
                                otf = apool.tile([P, TC], ST_DT, tag="otf",
                                                 name="otf", bufs=2)
                                # `split` > 1 pipelines the chain in
                                # column halves across DVE/Pool/ACT --
                                # used for the very last finalize, whose
                                # latency gates the final c_proj drain.
                                w = TC // split
                                for s_ in range(split):
                                    cs = slice(s_ * w, (s_ + 1) * w)
                                    qs = slice(qsl.start + s_ * w,
                                               qsl.start + (s_ + 1) * w)
                                    nc.vector.reciprocal(
                                        out=recip_row[:, cs],
                                        in_=rs_row[:, cs])
                                    nc.gpsimd.partition_broadcast(
                                        recipB[:, cs], recip_row[:, cs])
                                    nc.vector.tensor_mul(
                                        out=otf[:, cs], in0=ot_ps[:, cs],
                                        in1=recipB[:, cs])
                                    nc.vector.tensor_copy(
                                        out=otn_sb[:, 0, h, qs],
                                        in_=otf[:, cs])
                                    nc.vector.tensor_sub(
                                        out=otn_sb[:, 1, h, qs],
                                        in0=otf[:, cs],
                                        in1=otn_sb[:, 0, h, qs])

                            fin_pending[0] = finalize

                        # leftover ops from the previous qc, then the last
                        # head's finalize (before the next qc's rs16 memset),
                        # then queue this qc's c_proj for interleaving into
                        # the next qc's attention.
                        drain(10 ** 6)
                        fin_pending[0](split=4 if qc == NTC - 1 else 1)
                        fin_pending[0] = None
                        pending = list(cproj_gen(
                            qc,
                            use_act_fn=(lambda tb, j: (tb + j) % 2)
                            if qc == NTC - 1 else None))
                        pending.reverse()
                    drain(10 ** 6)

    nc.compile()
    return nc


_NC_CACHE = None


def _get_program():
    global _NC_CACHE
    if _NC_CACHE is None:
        _NC_CACHE = build_program()
    return _NC_CACHE


def _make_tables(pos):
    half = HD // 2
    inv_freq = 1.0 / (THETA ** (np.arange(half, dtype=np.float64) / half))
    ang = (pos + np.arange(T, dtype=np.float64))[None, :] * inv_freq[:, None]
    cos = np.cos(ang).astype(np.float32)
    sin = np.sin(ang).astype(np.float32)
    cosT = np.ascontiguousarray(np.concatenate([cos, cos], axis=0))
    sinT = np.ascontiguousarray(np.concatenate([-sin, sin], axis=0))
    return cosT, sinT


def _split8(a):
    """hi/lo fp8e4m3 decomposition: a ~= hi + lo (elementwise)."""
    a = np.asarray(a, dtype=np.float32)
    hi = a.astype(NP_F8)
    lo = (a - hi.astype(np.float32)).astype(NP_F8)
    return hi, lo


def make_in_maps(x, Wq, Wk, Wv, Wo, q_gamma, k_gamma, pos):
    x = np.asarray(x, dtype=np.float32)
    Wq = np.asarray(Wq, dtype=np.float32)
    Wk = np.asarray(Wk, dtype=np.float32)
    Wv = np.asarray(Wv, dtype=np.float32)
    Wo = np.asarray(Wo, dtype=np.float32)
    q_gamma = np.asarray(q_gamma, dtype=np.float32)
    k_gamma = np.asarray(k_gamma, dtype=np.float32)
    pos = int(np.asarray(pos))

    cosT, sinT = _make_tables(pos)
    tabs = np.ascontiguousarray(np.stack([cosT, sinT], axis=1)
                                .astype(NP_ST))
    gamma2 = np.ascontiguousarray((q_gamma * k_gamma).reshape(P, 1)
                                  .astype(np.float32))

    # x: per batch (D, T) -> hi/lo fp8 -> [P, NTC1, 2, NKT, TC1]
    x8 = []
    for b in range(B):
        hi, lo = _split8(x[b].T)
        st = np.stack([hi.reshape(NKT, P, NTC1, TC1),
                       lo.reshape(NKT, P, NTC1, TC1)], axis=0)
        x8.append(np.ascontiguousarray(st.transpose(2, 3, 0, 1, 4)))

    # weights pre-scaled by WS, split hi/lo, packed partition-major
    qh, ql = _split8(Wq.reshape(NKT, P, NKV, G, HD) * WS)
    kh, kl = _split8(Wk.reshape(NKT, P, NKV, HD) * WS)
    vh, vl = _split8(Wv.reshape(NKT, P, NKV, HD) * WS)
    oh, ol = _split8(Wo.reshape(NKV, G, P, D) * WS)

    in_maps = []
    for c in range(N_CORES):
        b, n = divmod(c, NKV)
        wq_p = np.stack([qh[:, :, n], ql[:, :, n]], axis=0)  # (2,NKT,P,G,HD)
        wk_p = np.stack([kh[:, :, n], kl[:, :, n]], axis=0)  # (2,NKT,P,HD)
        wv_p = np.stack([vh[:, :, n], vl[:, :, n]], axis=0)
        wo_p = np.stack([oh[n], ol[n]], axis=0)              # (2,G,P,D)
        in_maps.append({
            "xT8": x8[b],
            "wq8": np.ascontiguousarray(wq_p.transpose(2, 0, 3, 1, 4)),
            "wk8": np.ascontiguousarray(wk_p.transpose(2, 0, 1, 3)),
            "wv8": np.ascontiguousarray(wv_p.transpose(2, 0, 1, 3)),
            "wo8": np.ascontiguousarray(wo_p.transpose(2, 0, 1, 3)),
            "tabs": tabs,
            "gamma2": gamma2,
        })
    return in_maps


def kernel(x, Wq, Wk, Wv, Wo, q_gamma, k_gamma, pos):
    in_maps = make_in_maps(x, Wq, Wk, Wv, Wo, q_gamma, k_gamma, pos)
    nc = _get_program()
    res = bass_utils.run_bass_kernel_spmd(nc, in_maps,
                                          core_ids=list(range(N_CORES)))
    out = np.zeros((B, T, D), dtype=np.float32)
    for c in range(N_CORES):
        b = c // NKV
        out[b] += np.asarray(res.results[c]["y"], dtype=np.float32)
    return out


if __name__ == "__main__":
    build_program()
    print("program built OK")
